# revision 16
# baseline (speedup 1.0000x reference)
"""Multi-head attention (B=2, S=2048, D=1024, H=16) on 8 trn2 NeuronCores.

Sharding: core c -> batch b = c // 4, head-group g = c % 4 (4 heads/core).
Each core computes, for its batch and its 4 heads:
    qkv^T projection -> per-head scores^T = K Q^T / 8 -> exp -> AV with an
    appended ones-column on V (gives softmax denominators for free) ->
    normalize -> out^T [256, 2048].
Host transposes x per batch (so the contraction dim lands on partitions),
casts matmul operands to bf16, and transposes/scatters the per-core outputs
back into the full [2, 2048, 1024] f32 result.

All matmuls run in the standard 128x128 PE mode (operands are arranged so
every lhsT/rhs AP starts at partition 0 or is 128 partitions tall - avoids
the row-tiling encoding and its mode-switch drains).
"""

import os

import numpy as np
import ml_dtypes

import concourse.bass as bass
import concourse.mybir as mybir
from concourse.bass_utils import run_bass_kernel_spmd
from concourse.tile import TileContext
from concourse.vector_clock import ScopedClock

S = 2048          # sequence length
D = 1024          # embed dim
HPC = 4           # heads per core
DH = 64           # head dim
DQC = HPC * DH    # q/k/v channels per core (256)
NKT = D // 128    # contraction tiles for the projection (8)
NKC = S // 128    # key-position chunks (16)
NQB = S // 512    # query blocks (4)
VROW = DH + 1     # V' columns per head (64 values + ones column)

BF16 = mybir.dt.bfloat16
F32 = mybir.dt.float32
NPBF16 = ml_dtypes.bfloat16
ADD = mybir.AluOpType.add
MULT = mybir.AluOpType.mult


def _split_excess_waits(nc: bass.Bass, cap: int = 1) -> None:
    """The walrus build in this container supports at most one sync-wait
    command per ISA instruction; Tile attaches one wait per producer. Move
    excess waits onto same-engine NOPs inserted just before the offender
    (engine queues are FIFO, so the NOP waits gate everything behind them)."""
    n = 0
    for f in nc.m.functions:
        for blk in f.blocks:
            out = []
            for inst in blk.instructions:
                si = inst.sync_info
                waits = list(si.on_wait) if si is not None and si.on_wait else []
                if len(waits) > cap:
                    for w in waits[:-cap]:
                        n += 1
                        nop = mybir.InstNoOp(
                            name=f"{inst.name}-ws{n}", ins=[], outs=[]
                        )
                        nop.engine = inst.engine
                        nop.sync_info = mybir.SyncInfo(on_wait=[w], on_update=[])
                        out.append(nop)
                    inst.sync_info = mybir.SyncInfo(
                        on_wait=waits[-cap:],
                        on_update=list(si.on_update) if si.on_update else [],
                    )
                out.append(inst)
            blk.instructions = out


def _build_nc(split_waits: bool = True) -> bass.Bass:
    nc = bass.Bass()
    xT = nc.declare_dram_parameter("xT", [D, S], BF16, isOutput=False)
    wq = nc.declare_dram_parameter("wq", [D, DQC], BF16, isOutput=False)
    wk = nc.declare_dram_parameter("wk", [D, DQC], BF16, isOutput=False)
    wv = nc.declare_dram_parameter("wv", [D, DQC], BF16, isOutput=False)
    bq = nc.declare_dram_parameter("bq", [128, 2], F32, isOutput=False)
    bk = nc.declare_dram_parameter("bk", [128, 2], F32, isOutput=False)
    bv = nc.declare_dram_parameter("bv", [1, DQC], F32, isOutput=False)
    y = nc.declare_dram_parameter("y", [DQC, S], F32, isOutput=True)

    with TileContext(nc) as tc:
        with (
            tc.tile_pool(name="const", bufs=1) as cpool,
            tc.tile_pool(name="attn", bufs=8) as apool,
            tc.tile_pool(name="work", bufs=2) as wpool,
            tc.tile_pool(name="mm_ps", bufs=2, space="PSUM") as mmps,
            tc.tile_pool(name="av_ps", bufs=1, space="PSUM") as avps,
        ):
            # ---- input loads (weights/biases first: the first projection
            # matmuls need wq[0]/xts[0], not the whole xT) -------------------
            wsb = {}
            for name, dram in (("q", wq), ("k", wk), ("v", wv)):
                tiles = []
                for kt in range(NKT):
                    t = cpool.tile([128, DQC], BF16, tag=f"w{name}{kt}", name=f"w{name}{kt}")
                    nc.sync.dma_start(
                        out=t[:, :], in_=dram[kt * 128:(kt + 1) * 128, :]
                    )
                    tiles.append(t)
                wsb[name] = tiles

            bq_sb = cpool.tile([128, 2], F32, tag="bq")
            nc.sync.dma_start(out=bq_sb[:, :], in_=bq[:, :])
            bk_sb = cpool.tile([128, 2], F32, tag="bk")
            nc.sync.dma_start(out=bk_sb[:, :], in_=bk[:, :])
            bv_sb = cpool.tile([1, DQC], F32, tag="bv")
            nc.sync.dma_start(out=bv_sb[:, :], in_=bv[:, :])
            bvb = cpool.tile([128, DQC], F32, tag="bvb")
            nc.sync.dma_start(
                out=bvb[:, :],
                in_=bv_sb[0:1, :].unsqueeze(1).broadcast_to([1, 128, DQC]),
            )
            ones_col = cpool.tile([1, DH], F32, tag="ones_col")
            nc.vector.memset(ones_col[:, :], 1.0)

            xts = []
            for kt in range(NKT):
                t = cpool.tile([128, S], BF16, tag=f"xt{kt}", name=f"xt{kt}")
                nc.sync.dma_start(out=t[:, :], in_=xT[kt * 128:(kt + 1) * 128, :])
                xts.append(t)

            # V' tile: [128 (s-chunk partitions), 16 s-chunks x (4 heads x 65)]
            vp = cpool.tile([128, NKC * HPC * VROW], BF16, tag="vp")
            vp4 = vp[:, :].rearrange(
                "p (sc h c) -> p sc h c", sc=NKC, h=HPC, c=VROW
            )
            nc.vector.memset(vp4[:, :, :, DH:VROW], 1.0)

            # ---- qkv^T projection -------------------------------------------
            # Q^T, K^T chunk tiles: [128 (channel), 2048 (s)] x 2 chunks each,
            # emitted m-chunk-major so heads 0/1 are ready early.
            qth, kth = [None] * HPC, [None] * HPC
            qtc, ktc = [None, None], [None, None]
            for m in range(2):
                for which, chunks, wt, bias in (
                    ("q", qtc, wsb["q"], bq_sb),
                    ("k", ktc, wsb["k"], bk_sb),
                ):
                    chunk = cpool.tile([128, S], BF16, tag=f"{which}tc{m}", name=f"{which}tc{m}")
                    chunks[m] = chunk
                    for qb in range(NQB):
                        ps = mmps.tile([128, 512], F32, tag="mm", name=f"qkps_{which}{m}_{qb}")
                        for kt in range(NKT):
                            nc.tensor.matmul(
                                out=ps[:, :],
                                lhsT=wt[kt][:, m * 128:(m + 1) * 128],
                                rhs=xts[kt][:, qb * 512:(qb + 1) * 512],
                                start=(kt == 0),
                                stop=(kt == NKT - 1),
                            )
                        nc.vector.tensor_scalar(
                            out=chunk[:, qb * 512:(qb + 1) * 512],
                            in0=ps[:, :],
                            scalar1=bias[:, m:m + 1],
                            scalar2=None,
                            op0=ADD,
                        )
                # even head: direct slice; odd head: SBUF->SBUF DMA down to
                # partition base 0 (base-64 matmul APs would engage the
                # row-tiling encoding)
                qth[2 * m] = qtc[m][0:64, :]
                kth[2 * m] = ktc[m][0:64, :]
                h = 2 * m + 1
                qo = cpool.tile([64, S], BF16, tag=f"qto{h}", name=f"qto{h}")
                nc.sync.dma_start(out=qo[:, :], in_=qtc[m][64:128, :])
                qth[h] = qo[:, :]
                ko = cpool.tile([64, S], BF16, tag=f"kto{h}", name=f"kto{h}")
                nc.sync.dma_start(out=ko[:, :], in_=ktc[m][64:128, :])
                kth[h] = ko[:, :]

            # V (+bias) in [s, channel] layout, strided into V' head groups.
            for sc in range(NKC):
                ps = mmps.tile([128, DQC], F32, tag="mm", name=f"vps_{sc}")
                for kt in range(NKT):
                    nc.tensor.matmul(
                        out=ps[:, :],
                        lhsT=xts[kt][:, sc * 128:(sc + 1) * 128],
                        rhs=wsb["v"][kt][:, :],
                        start=(kt == 0),
                        stop=(kt == NKT - 1),
                    )
                nc.vector.tensor_tensor(
                    out=vp4[:, sc, :, 0:DH],
                    in0=ps[:, :].rearrange("p (h c) -> p h c", h=HPC),
                    in1=bvb[:, :].rearrange("p (h c) -> p h c", h=HPC),
                    op=ADD,
                )

            # ---- attention ---------------------------------------------------
            attn = [[None] * NKC for _ in range(HPC)]
            av_out = [None] * HPC

            def emit_scores(h, kc):
                at = apool.tile([128, S], BF16, tag="attn", name=f"attn{h}_{kc}")
                attn[h][kc] = at
                for half in range(2):
                    ps = mmps.tile([128, 1024], F32, tag="mm", name=f"sps_{h}_{kc}_{half}")
                    for j in range(2):
                        qb = 2 * half + j
                        nc.tensor.matmul(
                            out=ps[:, j * 512:(j + 1) * 512],
                            lhsT=kth[h][:, kc * 128:(kc + 1) * 128],
                            rhs=qth[h][:, qb * 512:(qb + 1) * 512],
                            start=True,
                            stop=True,
                        )
                    nc.scalar.activation(
                        out=at[:, half * 1024:(half + 1) * 1024],
                        in_=ps[:, :],
                        func=mybir.ActivationFunctionType.Exp,
                        scale=1.0 / np.sqrt(DH),
                    )

            def emit_av(h, kc):
                if kc == 0:
                    av_out[h] = avps.tile([VROW, S], F32, tag="av", name=f"av{h}")
                ps = av_out[h]
                for qb in range(NQB):
                    nc.tensor.matmul(
                        out=ps[:, qb * 512:(qb + 1) * 512],
                        lhsT=vp4[:, kc, h, :],
                        rhs=attn[h][kc][:, qb * 512:(qb + 1) * 512],
                        start=(kc == 0),
                        stop=(kc == NKC - 1),
                    )

            def emit_norm(h):
                ps = av_out[h]
                # Copy PSUM -> SBUF first so the single av_ps slot frees as
                # early as possible (the next head's AV matmuls wait on it).
                ot = wpool.tile([VROW, S], F32, tag="out", name=f"ot{h}")
                nc.vector.tensor_copy(ot[:, :], ps[:, :])
                # Reciprocal of the denominators (ones column of V' summed
                # into row DH). A [1, S] reciprocal runs on one DVE lane at
                # ~7.5 cyc/elem (~13us); scatter the row across 64 partitions
                # first so it takes S/DH elems per lane instead.
                rsc = wpool.tile([DH, S // DH], F32, tag="rsc", name=f"rsc{h}")
                nc.sync.dma_start(out=rsc[:, :], in_=ot[DH:VROW, :])
                nc.vector.reciprocal(out=rsc[:, :], in_=rsc[:, :])
                rrow = wpool.tile([1, S], F32, tag="rrow", name=f"rrow{h}")
                nc.sync.dma_start(out=rrow[:, :], in_=rsc[:, :])
                # Broadcast the reciprocal row to 64 partitions with the
                # tensor engine (ones[1,64]^T @ row), psum-bank sized chunks.
                for half in range(2):
                    rbp = mmps.tile(
                        [DH, 1024], F32, tag="mm", name=f"rb{h}_{half}"
                    )
                    for j in range(2):
                        sl = slice((2 * half + j) * 512, (2 * half + j + 1) * 512)
                        nc.tensor.matmul(
                            out=rbp[:, j * 512:(j + 1) * 512],
                            lhsT=ones_col[:, :],
                            rhs=rrow[:, sl],
                            start=True,
                            stop=True,
                        )
                    nc.vector.tensor_tensor(
                        out=ot[0:DH, half * 1024:(half + 1) * 1024],
                        in0=ot[0:DH, half * 1024:(half + 1) * 1024],
                        in1=rbp[:, :],
                        op=MULT,
                    )
                nc.sync.dma_start(out=y[h * DH:(h + 1) * DH, :], in_=ot[0:DH, :])

            AVLAG = 2
            for h in range(HPC):
                for kc in range(NKC):
                    emit_scores(h, kc)
                    if kc >= AVLAG:
                        emit_av(h, kc - AVLAG)
                for kc in range(NKC - AVLAG, NKC):
                    emit_av(h, kc)
                emit_norm(h)

    if split_waits:
        _split_excess_waits(nc)
    return nc


_NC = None


def _get_nc() -> bass.Bass:
    global _NC
    if _NC is None:
        _NC = _build_nc()
    return _NC


def make_in_maps(x, W_qkv, b_qkv):
    x = np.asarray(x, dtype=np.float32)
    W = np.asarray(W_qkv, dtype=np.float32)
    b = np.asarray(b_qkv, dtype=np.float32)
    in_maps = []
    for c in range(8):
        bi, g = divmod(c, 4)
        cols = slice(g * DQC, (g + 1) * DQC)
        xT = np.ascontiguousarray(x[bi].T).astype(NPBF16)
        m = {
            "xT": xT,
            "wq": np.ascontiguousarray(W[:, 0:D][:, cols]).astype(NPBF16),
            "wk": np.ascontiguousarray(W[:, D:2 * D][:, cols]).astype(NPBF16),
            "wv": np.ascontiguousarray(W[:, 2 * D:3 * D][:, cols]).astype(NPBF16),
            "bq": np.ascontiguousarray(b[0:D][cols].reshape(2, 128).T),
            "bk": np.ascontiguousarray(b[D:2 * D][cols].reshape(2, 128).T),
            "bv": b[2 * D:3 * D][cols].reshape(1, DQC).copy(),
        }
        in_maps.append(m)
    return in_maps


def gather_out(results):
    out = np.zeros((2, S, D), np.float32)
    for c in range(8):
        bi, g = divmod(c, 4)
        out[bi, :, g * DQC:(g + 1) * DQC] = np.asarray(
            results[c]["y"], np.float32
        ).T
    return out


def kernel(x, W_qkv, b_qkv):
    nc = _get_nc()
    in_maps = make_in_maps(x, W_qkv, b_qkv)
    trace = bool(int(os.environ.get("BASS_KERNEL_TRACE", "0")))
    res = run_bass_kernel_spmd(nc, in_maps, list(range(8)), trace=trace)
    if trace:
        kernel.last_result = res
    return gather_out(res.results)


# revision 18
# speedup vs baseline: 1.1995x; 1.1995x over previous
"""Multi-head attention (B=2, S=2048, D=1024, H=16) on 8 trn2 NeuronCores.

Sharding: core c -> batch b = c // 4, head-group g = c % 4 (4 heads/core).
Each core computes, for its batch and its 4 heads:
    qkv^T projection -> per-head scores^T = K Q^T / 8 -> exp -> AV with an
    appended ones-column on V (gives softmax denominators for free) ->
    normalize -> out^T [256, 2048].
Host transposes x per batch (so the contraction dim lands on partitions),
casts matmul operands to bf16, and transposes/scatters the per-core outputs
back into the full [2, 2048, 1024] f32 result.

All matmuls run in the standard 128x128 PE mode (operands are arranged so
every lhsT/rhs AP starts at partition 0 or is 128 partitions tall - avoids
the row-tiling encoding and its mode-switch drains).
"""

import os

import numpy as np
import ml_dtypes

import concourse.bass as bass
import concourse.mybir as mybir
from concourse.bass_utils import run_bass_kernel_spmd
from concourse.tile import TileContext
from concourse.vector_clock import ScopedClock

S = 2048          # sequence length
D = 1024          # embed dim
HPC = 4           # heads per core
DH = 64           # head dim
DQC = HPC * DH    # q/k/v channels per core (256)
NKT = D // 128    # contraction tiles for the projection (8)
NKC = S // 128    # key-position chunks (16)
NQB = S // 512    # query blocks (4)
VROW = DH + 1     # V' columns per head (64 values + ones column)

BF16 = mybir.dt.bfloat16
F32 = mybir.dt.float32
NPBF16 = ml_dtypes.bfloat16
ADD = mybir.AluOpType.add
MULT = mybir.AluOpType.mult


def _split_excess_waits(nc: bass.Bass, cap: int = 1) -> None:
    """The walrus build in this container supports at most one sync-wait
    command per ISA instruction; Tile attaches one wait per producer. Move
    excess waits onto same-engine NOPs inserted just before the offender
    (engine queues are FIFO, so the NOP waits gate everything behind them)."""
    n = 0
    for f in nc.m.functions:
        for blk in f.blocks:
            out = []
            for inst in blk.instructions:
                si = inst.sync_info
                waits = list(si.on_wait) if si is not None and si.on_wait else []
                if len(waits) > cap:
                    for w in waits[:-cap]:
                        n += 1
                        nop = mybir.InstNoOp(
                            name=f"{inst.name}-ws{n}", ins=[], outs=[]
                        )
                        nop.engine = inst.engine
                        nop.sync_info = mybir.SyncInfo(on_wait=[w], on_update=[])
                        out.append(nop)
                    inst.sync_info = mybir.SyncInfo(
                        on_wait=waits[-cap:],
                        on_update=list(si.on_update) if si.on_update else [],
                    )
                out.append(inst)
            blk.instructions = out


def _build_nc(split_waits: bool = True) -> bass.Bass:
    nc = bass.Bass()
    xT = nc.declare_dram_parameter("xT", [D, S], BF16, isOutput=False)
    wq = nc.declare_dram_parameter("wq", [D, DQC], BF16, isOutput=False)
    wk = nc.declare_dram_parameter("wk", [D, DQC], BF16, isOutput=False)
    wv = nc.declare_dram_parameter("wv", [D, DQC], BF16, isOutput=False)
    bq = nc.declare_dram_parameter("bq", [128, 2], F32, isOutput=False)
    bk = nc.declare_dram_parameter("bk", [128, 2], F32, isOutput=False)
    bv = nc.declare_dram_parameter("bv", [1, DQC], F32, isOutput=False)
    y = nc.declare_dram_parameter("y", [DQC, S], F32, isOutput=True)

    with TileContext(nc) as tc:
        with (
            tc.tile_pool(name="const", bufs=1) as cpool,
            tc.tile_pool(name="attn", bufs=16) as apool,
            tc.tile_pool(name="work", bufs=2) as wpool,
            tc.tile_pool(name="mm_ps", bufs=2, space="PSUM") as mmps,
            tc.tile_pool(name="av_ps", bufs=1, space="PSUM") as avps,
        ):
            # ---- input loads (weights/biases first: the first projection
            # matmuls need wq[0]/xts[0], not the whole xT) -------------------
            wsb = {}
            for name, dram in (("q", wq), ("k", wk), ("v", wv)):
                t = cpool.tile([128, NKT * DQC], BF16, tag=f"w{name}", name=f"w{name}")
                nc.sync.dma_start(
                    out=t[:, :],
                    in_=dram[:, :].rearrange("(kt p) c -> p kt c", p=128),
                )
                wsb[name] = t[:, :].rearrange("p (kt c) -> p kt c", kt=NKT)

            bq_sb = cpool.tile([128, 2], F32, tag="bq")
            nc.sync.dma_start(out=bq_sb[:, :], in_=bq[:, :])
            bk_sb = cpool.tile([128, 2], F32, tag="bk")
            nc.sync.dma_start(out=bk_sb[:, :], in_=bk[:, :])
            bv_sb = cpool.tile([1, DQC], F32, tag="bv")
            nc.sync.dma_start(out=bv_sb[:, :], in_=bv[:, :])
            ones_row = cpool.tile([1, 128], F32, tag="ones_row")
            nc.vector.memset(ones_row[:, :], 1.0)
            # bias-v broadcast to all partitions via the tensor engine
            bvb_ps = mmps.tile([128, DQC], F32, tag="mm", name="bvb_ps")
            nc.tensor.matmul(
                out=bvb_ps[:, :], lhsT=ones_row[:, :], rhs=bv_sb[:, :],
                start=True, stop=True,
            )
            bvb = cpool.tile([128, DQC], F32, tag="bvb")
            nc.vector.tensor_copy(bvb[:, :], bvb_ps[:, :])

            xts = []
            for kt in range(NKT):
                t = cpool.tile([128, S], BF16, tag=f"xt{kt}", name=f"xt{kt}")
                nc.sync.dma_start(out=t[:, :], in_=xT[kt * 128:(kt + 1) * 128, :])
                xts.append(t)

            # V' tile: [128 (s-chunk partitions), 16 s-chunks x (4 heads x 65)]
            vp = cpool.tile([128, NKC * HPC * VROW], BF16, tag="vp")
            vp4 = vp[:, :].rearrange(
                "p (sc h c) -> p sc h c", sc=NKC, h=HPC, c=VROW
            )
            nc.vector.memset(vp4[:, :, :, DH:VROW], 1.0)

            # ---- qkv^T projection -------------------------------------------
            # Q^T, K^T chunk tiles: [128 (channel), 2048 (s)] x 2 chunks each,
            # emitted m-chunk-major so heads 0/1 are ready early.
            qth, kth = [None] * HPC, [None] * HPC
            qtc, ktc = [None, None], [None, None]
            for m in range(2):
                for which, chunks, wt, bias in (
                    ("q", qtc, wsb["q"], bq_sb),
                    ("k", ktc, wsb["k"], bk_sb),
                ):
                    chunk = cpool.tile([128, S], BF16, tag=f"{which}tc{m}", name=f"{which}tc{m}")
                    chunks[m] = chunk
                    for qb in range(NQB):
                        ps = mmps.tile([128, 512], F32, tag="mm", name=f"qkps_{which}{m}_{qb}")
                        for kt in range(NKT):
                            nc.tensor.matmul(
                                out=ps[:, :],
                                lhsT=wt[:, kt, m * 128:(m + 1) * 128],
                                rhs=xts[kt][:, qb * 512:(qb + 1) * 512],
                                start=(kt == 0),
                                stop=(kt == NKT - 1),
                            )
                        nc.vector.tensor_scalar(
                            out=chunk[:, qb * 512:(qb + 1) * 512],
                            in0=ps[:, :],
                            scalar1=bias[:, m:m + 1],
                            scalar2=None,
                            op0=ADD,
                        )
                # even head: direct slice; odd head: SBUF->SBUF DMA down to
                # partition base 0 (base-64 matmul APs would engage the
                # row-tiling encoding)
                qth[2 * m] = qtc[m][0:64, :]
                kth[2 * m] = ktc[m][0:64, :]
                h = 2 * m + 1
                qo = cpool.tile([64, S], BF16, tag=f"qto{h}", name=f"qto{h}")
                nc.sync.dma_start(out=qo[:, :], in_=qtc[m][64:128, :])
                qth[h] = qo[:, :]
                ko = cpool.tile([64, S], BF16, tag=f"kto{h}", name=f"kto{h}")
                nc.sync.dma_start(out=ko[:, :], in_=ktc[m][64:128, :])
                kth[h] = ko[:, :]

            # V (+bias) in [s, channel] layout, strided into V' head groups.
            for sc in range(NKC):
                ps = mmps.tile([128, DQC], F32, tag="mm", name=f"vps_{sc}")
                for kt in range(NKT):
                    nc.tensor.matmul(
                        out=ps[:, :],
                        lhsT=xts[kt][:, sc * 128:(sc + 1) * 128],
                        rhs=wsb["v"][:, kt, :],
                        start=(kt == 0),
                        stop=(kt == NKT - 1),
                    )
                nc.vector.tensor_tensor(
                    out=vp4[:, sc, :, 0:DH],
                    in0=ps[:, :].rearrange("p (h c) -> p h c", h=HPC),
                    in1=bvb[:, :].rearrange("p (h c) -> p h c", h=HPC),
                    op=ADD,
                )

            # ---- attention ---------------------------------------------------
            attn = [[None] * NKC for _ in range(HPC)]
            av_out = [None] * HPC

            def emit_scores(h, kc):
                at = apool.tile([128, S], BF16, tag="attn", name=f"attn{h}_{kc}")
                attn[h][kc] = at
                for half in range(2):
                    ps = mmps.tile([128, 1024], F32, tag="mm", name=f"sps_{h}_{kc}_{half}")
                    for j in range(2):
                        qb = 2 * half + j
                        nc.tensor.matmul(
                            out=ps[:, j * 512:(j + 1) * 512],
                            lhsT=kth[h][:, kc * 128:(kc + 1) * 128],
                            rhs=qth[h][:, qb * 512:(qb + 1) * 512],
                            start=True,
                            stop=True,
                        )
                    nc.scalar.activation(
                        out=at[:, half * 1024:(half + 1) * 1024],
                        in_=ps[:, :],
                        func=mybir.ActivationFunctionType.Exp,
                        scale=1.0 / np.sqrt(DH),
                    )

            def emit_av(h, kc):
                if kc == 0:
                    av_out[h] = avps.tile([VROW, S], F32, tag="av", name=f"av{h}")
                ps = av_out[h]
                for qb in range(NQB):
                    nc.tensor.matmul(
                        out=ps[:, qb * 512:(qb + 1) * 512],
                        lhsT=vp4[:, kc, h, :],
                        rhs=attn[h][kc][:, qb * 512:(qb + 1) * 512],
                        start=(kc == 0),
                        stop=(kc == NKC - 1),
                    )

            def emit_norm(h):
                ps = av_out[h]
                # Copy PSUM -> SBUF first so the single av_ps slot frees as
                # early as possible (the next head's AV matmuls wait on it).
                ot = wpool.tile([VROW, S], F32, tag="out", name=f"ot{h}")
                nc.vector.tensor_copy(ot[:, :], ps[:, :])
                # Reciprocal of the denominators (ones column of V' summed
                # into row DH). A [1, S] reciprocal runs on one DVE lane at
                # ~7.5 cyc/elem (~13us); scatter the row across 64 partitions
                # first so it takes S/DH elems per lane instead.
                rsc = wpool.tile([DH, S // DH], F32, tag="rsc", name=f"rsc{h}")
                nc.sync.dma_start(out=rsc[:, :], in_=ot[DH:VROW, :])
                nc.vector.reciprocal(out=rsc[:, :], in_=rsc[:, :])
                rrow = wpool.tile([1, S], F32, tag="rrow", name=f"rrow{h}")
                nc.sync.dma_start(out=rrow[:, :], in_=rsc[:, :])
                # Broadcast the reciprocal row to 64 partitions with the
                # tensor engine (ones[1,64]^T @ row), psum-bank sized chunks.
                for half in range(2):
                    rbp = mmps.tile(
                        [DH, 1024], F32, tag="mm", name=f"rb{h}_{half}"
                    )
                    for j in range(2):
                        sl = slice((2 * half + j) * 512, (2 * half + j + 1) * 512)
                        nc.tensor.matmul(
                            out=rbp[:, j * 512:(j + 1) * 512],
                            lhsT=ones_row[:, 0:DH],
                            rhs=rrow[:, sl],
                            start=True,
                            stop=True,
                        )
                    nc.vector.tensor_tensor(
                        out=ot[0:DH, half * 1024:(half + 1) * 1024],
                        in0=ot[0:DH, half * 1024:(half + 1) * 1024],
                        in1=rbp[:, :],
                        op=MULT,
                    )
                nc.sync.dma_start(out=y[h * DH:(h + 1) * DH, :], in_=ot[0:DH, :])

            for h in range(HPC):
                for kc in range(NKC):
                    emit_scores(h, kc)
                    if h > 0:
                        emit_av(h - 1, kc)
                if h > 0:
                    emit_norm(h - 1)
            for kc in range(NKC):
                emit_av(HPC - 1, kc)
            emit_norm(HPC - 1)

    if split_waits:
        _split_excess_waits(nc)
    return nc


_NC = None


def _get_nc() -> bass.Bass:
    global _NC
    if _NC is None:
        _NC = _build_nc()
    return _NC


def make_in_maps(x, W_qkv, b_qkv):
    x = np.asarray(x, dtype=np.float32)
    W = np.asarray(W_qkv, dtype=np.float32)
    b = np.asarray(b_qkv, dtype=np.float32)
    in_maps = []
    for c in range(8):
        bi, g = divmod(c, 4)
        cols = slice(g * DQC, (g + 1) * DQC)
        xT = np.ascontiguousarray(x[bi].T).astype(NPBF16)
        m = {
            "xT": xT,
            "wq": np.ascontiguousarray(W[:, 0:D][:, cols]).astype(NPBF16),
            "wk": np.ascontiguousarray(W[:, D:2 * D][:, cols]).astype(NPBF16),
            "wv": np.ascontiguousarray(W[:, 2 * D:3 * D][:, cols]).astype(NPBF16),
            "bq": np.ascontiguousarray(b[0:D][cols].reshape(2, 128).T),
            "bk": np.ascontiguousarray(b[D:2 * D][cols].reshape(2, 128).T),
            "bv": b[2 * D:3 * D][cols].reshape(1, DQC).copy(),
        }
        in_maps.append(m)
    return in_maps


def gather_out(results):
    out = np.zeros((2, S, D), np.float32)
    for c in range(8):
        bi, g = divmod(c, 4)
        out[bi, :, g * DQC:(g + 1) * DQC] = np.asarray(
            results[c]["y"], np.float32
        ).T
    return out


def kernel(x, W_qkv, b_qkv):
    nc = _get_nc()
    in_maps = make_in_maps(x, W_qkv, b_qkv)
    trace = bool(int(os.environ.get("BASS_KERNEL_TRACE", "0")))
    res = run_bass_kernel_spmd(nc, in_maps, list(range(8)), trace=trace)
    if trace:
        kernel.last_result = res
    return gather_out(res.results)


# revision 21
# speedup vs baseline: 1.2810x; 1.0679x over previous
"""Multi-head attention (B=2, S=2048, D=1024, H=16) on 8 trn2 NeuronCores.

Sharding: core c -> batch b = c // 4, head-group g = c % 4 (4 heads/core).
Each core computes, for its batch and its 4 heads:
    qkv^T projection -> per-head scores^T = K Q^T / 8 -> exp -> AV with an
    appended ones-column on V (gives softmax denominators for free) ->
    normalize -> out^T [256, 2048].
Host transposes x per batch (so the contraction dim lands on partitions),
casts matmul operands to bf16, and transposes/scatters the per-core outputs
back into the full [2, 2048, 1024] f32 result.

All matmuls run in the standard 128x128 PE mode (operands are arranged so
every lhsT/rhs AP starts at partition 0 or is 128 partitions tall - avoids
the row-tiling encoding and its mode-switch drains).
"""

import os

import numpy as np
import ml_dtypes

import concourse.bass as bass
import concourse.mybir as mybir
from concourse.bass_utils import run_bass_kernel_spmd
from concourse.tile import TileContext
from concourse.vector_clock import ScopedClock

S = 2048          # sequence length
D = 1024          # embed dim
HPC = 4           # heads per core
DH = 64           # head dim
DQC = HPC * DH    # q/k/v channels per core (256)
NKT = D // 128    # contraction tiles for the projection (8)
NKC = S // 128    # key-position chunks (16)
NQB = S // 512    # query blocks (4)
VROW = DH + 1     # V' columns per head (64 values + ones column)

BF16 = mybir.dt.bfloat16
F32 = mybir.dt.float32
F32R = mybir.dt.float32r
NPBF16 = ml_dtypes.bfloat16
ADD = mybir.AluOpType.add
MULT = mybir.AluOpType.mult


def _split_excess_waits(nc: bass.Bass, cap: int = 1) -> None:
    """The walrus build in this container supports at most one sync-wait
    command per ISA instruction; Tile attaches one wait per producer. Move
    excess waits onto same-engine NOPs inserted just before the offender
    (engine queues are FIFO, so the NOP waits gate everything behind them)."""
    n = 0
    for f in nc.m.functions:
        for blk in f.blocks:
            out = []
            for inst in blk.instructions:
                si = inst.sync_info
                waits = list(si.on_wait) if si is not None and si.on_wait else []
                if len(waits) > cap:
                    for w in waits[:-cap]:
                        n += 1
                        nop = mybir.InstNoOp(
                            name=f"{inst.name}-ws{n}", ins=[], outs=[]
                        )
                        nop.engine = inst.engine
                        nop.sync_info = mybir.SyncInfo(on_wait=[w], on_update=[])
                        out.append(nop)
                    inst.sync_info = mybir.SyncInfo(
                        on_wait=waits[-cap:],
                        on_update=list(si.on_update) if si.on_update else [],
                    )
                out.append(inst)
            blk.instructions = out


def _build_nc(split_waits: bool = True) -> bass.Bass:
    nc = bass.Bass()
    xT = nc.declare_dram_parameter("xT", [D, S], BF16, isOutput=False)
    wq = nc.declare_dram_parameter("wq", [D, DQC], BF16, isOutput=False)
    wk = nc.declare_dram_parameter("wk", [D, DQC], BF16, isOutput=False)
    wv = nc.declare_dram_parameter("wv", [D, DQC], BF16, isOutput=False)
    bq = nc.declare_dram_parameter("bq", [128, 2], F32, isOutput=False)
    bk = nc.declare_dram_parameter("bk", [128, 2], F32, isOutput=False)
    bv = nc.declare_dram_parameter("bv", [1, DQC], F32R, isOutput=False)
    y = nc.declare_dram_parameter("y", [DQC, S], F32, isOutput=True)

    with TileContext(nc) as tc:
        with (
            tc.tile_pool(name="const", bufs=1) as cpool,
            tc.tile_pool(name="attn", bufs=16) as apool,
            tc.tile_pool(name="work", bufs=2) as wpool,
            tc.tile_pool(name="mm_ps", bufs=2, space="PSUM") as mmps,
            tc.tile_pool(name="av_ps", bufs=1, space="PSUM") as avps,
        ):
            # ---- input loads (weights/biases first: the first projection
            # matmuls need wq[0]/xts[0], not the whole xT) -------------------
            wsb = {}
            for name, dram in (("q", wq), ("k", wk), ("v", wv)):
                t = cpool.tile([128, NKT * DQC], BF16, tag=f"w{name}", name=f"w{name}")
                nc.sync.dma_start(
                    out=t[:, :],
                    in_=dram[:, :].rearrange("(kt p) c -> p kt c", p=128),
                )
                wsb[name] = t[:, :].rearrange("p (kt c) -> p kt c", kt=NKT)

            bq_sb = cpool.tile([128, 2], F32, tag="bq")
            nc.sync.dma_start(out=bq_sb[:, :], in_=bq[:, :])
            bk_sb = cpool.tile([128, 2], F32, tag="bk")
            nc.sync.dma_start(out=bk_sb[:, :], in_=bk[:, :])
            bv_sb = cpool.tile([1, DQC], F32R, tag="bv")
            nc.sync.dma_start(out=bv_sb[:, :], in_=bv[:, :])
            ones_row = cpool.tile([1, 128], F32R, tag="ones_row")
            nc.vector.memset(
                ones_row[:, :].bitcast(mybir.dt.uint32), 0x3F800000
            )
            # bias-v broadcast to all partitions via the tensor engine
            bvb_ps = mmps.tile([128, DQC], F32, tag="mm", name="bvb_ps")
            nc.tensor.matmul(
                out=bvb_ps[:, :], lhsT=ones_row[:, :], rhs=bv_sb[:, :],
                start=True, stop=True,
            )
            bvb = cpool.tile([128, DQC], F32, tag="bvb")
            nc.vector.tensor_copy(bvb[:, :], bvb_ps[:, :])

            xts = []
            for kt in range(NKT):
                t = cpool.tile([128, S], BF16, tag=f"xt{kt}", name=f"xt{kt}")
                nc.sync.dma_start(out=t[:, :], in_=xT[kt * 128:(kt + 1) * 128, :])
                xts.append(t)

            # V' tile: [128 (s-chunk partitions), 16 s-chunks x (4 heads x 65)]
            vp = cpool.tile([128, NKC * HPC * VROW], BF16, tag="vp")
            vp4 = vp[:, :].rearrange(
                "p (sc h c) -> p sc h c", sc=NKC, h=HPC, c=VROW
            )
            nc.vector.memset(vp4[:, :, :, DH:VROW], 1.0)

            # ---- qkv^T projection -------------------------------------------
            # Q^T, K^T chunk tiles: [128 (channel), 2048 (s)] x 2 chunks each,
            # emitted m-chunk-major so heads 0/1 are ready early.
            qth, kth = [None] * HPC, [None] * HPC
            qtc, ktc = [None, None], [None, None]
            for m in range(2):
                for which, chunks, wt, bias in (
                    ("q", qtc, wsb["q"], bq_sb),
                    ("k", ktc, wsb["k"], bk_sb),
                ):
                    chunk = cpool.tile([128, S], BF16, tag=f"{which}tc{m}", name=f"{which}tc{m}")
                    chunks[m] = chunk
                    for qb in range(NQB):
                        ps = mmps.tile([128, 512], F32, tag="mm", name=f"qkps_{which}{m}_{qb}")
                        for kt in range(NKT):
                            nc.tensor.matmul(
                                out=ps[:, :],
                                lhsT=wt[:, kt, m * 128:(m + 1) * 128],
                                rhs=xts[kt][:, qb * 512:(qb + 1) * 512],
                                start=(kt == 0),
                                stop=(kt == NKT - 1),
                            )
                        nc.vector.tensor_scalar(
                            out=chunk[:, qb * 512:(qb + 1) * 512],
                            in0=ps[:, :],
                            scalar1=bias[:, m:m + 1],
                            scalar2=None,
                            op0=ADD,
                        )
                # even head: direct slice; odd head: SBUF->SBUF DMA down to
                # partition base 0 (base-64 matmul APs would engage the
                # row-tiling encoding)
                qth[2 * m] = qtc[m][0:64, :]
                kth[2 * m] = ktc[m][0:64, :]
                h = 2 * m + 1
                qo = cpool.tile([64, S], BF16, tag=f"qto{h}", name=f"qto{h}")
                nc.sync.dma_start(out=qo[:, :], in_=qtc[m][64:128, :])
                qth[h] = qo[:, :]
                ko = cpool.tile([64, S], BF16, tag=f"kto{h}", name=f"kto{h}")
                nc.sync.dma_start(out=ko[:, :], in_=ktc[m][64:128, :])
                kth[h] = ko[:, :]

            # V (+bias) in [s, channel] layout, strided into V' head groups.
            for sc in range(NKC):
                ps = mmps.tile([128, DQC], F32, tag="mm", name=f"vps_{sc}")
                for kt in range(NKT):
                    nc.tensor.matmul(
                        out=ps[:, :],
                        lhsT=xts[kt][:, sc * 128:(sc + 1) * 128],
                        rhs=wsb["v"][:, kt, :],
                        start=(kt == 0),
                        stop=(kt == NKT - 1),
                    )
                nc.vector.tensor_tensor(
                    out=vp4[:, sc, :, 0:DH],
                    in0=ps[:, :].rearrange("p (h c) -> p h c", h=HPC),
                    in1=bvb[:, :].rearrange("p (h c) -> p h c", h=HPC),
                    op=ADD,
                )

            # ---- attention ---------------------------------------------------
            attn = [[None] * NKC for _ in range(HPC)]
            av_out = [None] * HPC

            def emit_scores(h, kc):
                at = apool.tile([128, S], BF16, tag="attn", name=f"attn{h}_{kc}")
                attn[h][kc] = at
                for half in range(2):
                    ps = mmps.tile([128, 1024], F32, tag="mm", name=f"sps_{h}_{kc}_{half}")
                    for j in range(2):
                        qb = 2 * half + j
                        nc.tensor.matmul(
                            out=ps[:, j * 512:(j + 1) * 512],
                            lhsT=kth[h][:, kc * 128:(kc + 1) * 128],
                            rhs=qth[h][:, qb * 512:(qb + 1) * 512],
                            start=True,
                            stop=True,
                        )
                    nc.scalar.activation(
                        out=at[:, half * 1024:(half + 1) * 1024],
                        in_=ps[:, :],
                        func=mybir.ActivationFunctionType.Exp,
                        scale=1.0 / np.sqrt(DH),
                    )

            def emit_av(h, kc):
                if kc == 0:
                    av_out[h] = avps.tile([VROW, S], F32, tag="av", name=f"av{h}")
                ps = av_out[h]
                for qb in range(NQB):
                    nc.tensor.matmul(
                        out=ps[:, qb * 512:(qb + 1) * 512],
                        lhsT=vp4[:, kc, h, :],
                        rhs=attn[h][kc][:, qb * 512:(qb + 1) * 512],
                        start=(kc == 0),
                        stop=(kc == NKC - 1),
                    )

            def emit_norm(h):
                ps = av_out[h]
                # Copy PSUM -> SBUF first so the single av_ps slot frees as
                # early as possible (the next head's AV matmuls wait on it).
                ot = wpool.tile([VROW, S], F32, tag="out", name=f"ot{h}")
                nc.vector.tensor_copy(ot[:, :], ps[:, :])
                # Reciprocal of the denominators (ones column of V' summed
                # into row DH). A [1, S] reciprocal runs on one DVE lane at
                # ~7.5 cyc/elem (~13us); scatter the row across 64 partitions
                # first so it takes S/DH elems per lane instead.
                rsc = wpool.tile([DH, S // DH], F32, tag="rsc", name=f"rsc{h}")
                nc.sync.dma_start(out=rsc[:, :], in_=ot[DH:VROW, :])
                rscr = wpool.tile([DH, S // DH], F32R, tag="rscr", name=f"rscr{h}")
                with nc.allow_low_precision(reason="f32r feed for broadcast matmul"):
                    nc.vector.reciprocal(out=rscr[:, :], in_=rsc[:, :])
                rrow = wpool.tile([1, S], F32R, tag="rrow", name=f"rrow{h}")
                nc.sync.dma_start(out=rrow[:, :], in_=rscr[:, :])
                # Broadcast the reciprocal row to 64 partitions with the
                # tensor engine (ones[1,64]^T @ row), psum-bank sized chunks.
                for half in range(2):
                    rbp = mmps.tile(
                        [DH, 1024], F32, tag="mm", name=f"rb{h}_{half}"
                    )
                    for j in range(2):
                        sl = slice((2 * half + j) * 512, (2 * half + j + 1) * 512)
                        nc.tensor.matmul(
                            out=rbp[:, j * 512:(j + 1) * 512],
                            lhsT=ones_row[:, 0:DH],
                            rhs=rrow[:, sl],
                            start=True,
                            stop=True,
                        )
                    nc.vector.tensor_tensor(
                        out=ot[0:DH, half * 1024:(half + 1) * 1024],
                        in0=ot[0:DH, half * 1024:(half + 1) * 1024],
                        in1=rbp[:, :],
                        op=MULT,
                    )
                nc.sync.dma_start(out=y[h * DH:(h + 1) * DH, :], in_=ot[0:DH, :])

            for h in range(HPC):
                for kc in range(NKC):
                    emit_scores(h, kc)
                    if h > 0:
                        emit_av(h - 1, kc)
                if h > 0:
                    emit_norm(h - 1)
            for kc in range(NKC):
                emit_av(HPC - 1, kc)
            emit_norm(HPC - 1)

    if split_waits:
        _split_excess_waits(nc)
    return nc


_NC = None


def _get_nc() -> bass.Bass:
    global _NC
    if _NC is None:
        _NC = _build_nc()
    return _NC


def make_in_maps(x, W_qkv, b_qkv):
    x = np.asarray(x, dtype=np.float32)
    W = np.asarray(W_qkv, dtype=np.float32)
    b = np.asarray(b_qkv, dtype=np.float32)
    in_maps = []
    for c in range(8):
        bi, g = divmod(c, 4)
        cols = slice(g * DQC, (g + 1) * DQC)
        xT = np.ascontiguousarray(x[bi].T).astype(NPBF16)
        m = {
            "xT": xT,
            "wq": np.ascontiguousarray(W[:, 0:D][:, cols]).astype(NPBF16),
            "wk": np.ascontiguousarray(W[:, D:2 * D][:, cols]).astype(NPBF16),
            "wv": np.ascontiguousarray(W[:, 2 * D:3 * D][:, cols]).astype(NPBF16),
            "bq": np.ascontiguousarray(b[0:D][cols].reshape(2, 128).T),
            "bk": np.ascontiguousarray(b[D:2 * D][cols].reshape(2, 128).T),
            "bv": b[2 * D:3 * D][cols].reshape(1, DQC).copy(),
        }
        in_maps.append(m)
    return in_maps


def gather_out(results):
    out = np.zeros((2, S, D), np.float32)
    for c in range(8):
        bi, g = divmod(c, 4)
        out[bi, :, g * DQC:(g + 1) * DQC] = np.asarray(
            results[c]["y"], np.float32
        ).T
    return out


def kernel(x, W_qkv, b_qkv):
    nc = _get_nc()
    in_maps = make_in_maps(x, W_qkv, b_qkv)
    trace = bool(int(os.environ.get("BASS_KERNEL_TRACE", "0")))
    res = run_bass_kernel_spmd(nc, in_maps, list(range(8)), trace=trace)
    if trace:
        kernel.last_result = res
    return gather_out(res.results)


# revision 22
# speedup vs baseline: 1.3863x; 1.0823x over previous
"""Multi-head attention (B=2, S=2048, D=1024, H=16) on 8 trn2 NeuronCores.

Sharding: core c -> batch b = c // 4, head-group g = c % 4 (4 heads/core).
Each core computes, for its batch and its 4 heads:
    qkv^T projection -> per-head scores^T = K Q^T / 8 -> exp -> AV with an
    appended ones-column on V (gives softmax denominators for free) ->
    normalize -> out^T [256, 2048].
Host transposes x per batch (so the contraction dim lands on partitions),
casts matmul operands to bf16, and transposes/scatters the per-core outputs
back into the full [2, 2048, 1024] f32 result.

All matmuls run in the standard 128x128 PE mode (operands are arranged so
every lhsT/rhs AP starts at partition 0 or is 128 partitions tall - avoids
the row-tiling encoding and its mode-switch drains).
"""

import os

import numpy as np
import ml_dtypes

import concourse.bass as bass
import concourse.mybir as mybir
from concourse.bass_utils import run_bass_kernel_spmd
from concourse.tile import TileContext
from concourse.vector_clock import ScopedClock

S = 2048          # sequence length
D = 1024          # embed dim
HPC = 4           # heads per core
DH = 64           # head dim
DQC = HPC * DH    # q/k/v channels per core (256)
NKT = D // 128    # contraction tiles for the projection (8)
NKC = S // 128    # key-position chunks (16)
NQB = S // 512    # query blocks (4)
VROW = DH + 1     # V' columns per head (64 values + ones column)

BF16 = mybir.dt.bfloat16
F32 = mybir.dt.float32
F32R = mybir.dt.float32r
NPBF16 = ml_dtypes.bfloat16
ADD = mybir.AluOpType.add
MULT = mybir.AluOpType.mult


def _split_excess_waits(nc: bass.Bass, cap: int = 1) -> None:
    """The walrus build in this container supports at most one sync-wait
    command per ISA instruction; Tile attaches one wait per producer. Move
    excess waits onto same-engine NOPs inserted just before the offender
    (engine queues are FIFO, so the NOP waits gate everything behind them)."""
    n = 0
    for f in nc.m.functions:
        for blk in f.blocks:
            out = []
            for inst in blk.instructions:
                si = inst.sync_info
                waits = list(si.on_wait) if si is not None and si.on_wait else []
                if len(waits) > cap:
                    for w in waits[:-cap]:
                        n += 1
                        nop = mybir.InstNoOp(
                            name=f"{inst.name}-ws{n}", ins=[], outs=[]
                        )
                        nop.engine = inst.engine
                        nop.sync_info = mybir.SyncInfo(on_wait=[w], on_update=[])
                        out.append(nop)
                    inst.sync_info = mybir.SyncInfo(
                        on_wait=waits[-cap:],
                        on_update=list(si.on_update) if si.on_update else [],
                    )
                out.append(inst)
            blk.instructions = out


def _build_nc(split_waits: bool = True) -> bass.Bass:
    nc = bass.Bass()
    xT = nc.declare_dram_parameter("xT", [D, S], BF16, isOutput=False)
    wq = nc.declare_dram_parameter("wq", [D, DQC], BF16, isOutput=False)
    wk = nc.declare_dram_parameter("wk", [D, DQC], BF16, isOutput=False)
    wv = nc.declare_dram_parameter("wv", [D, DQC], BF16, isOutput=False)
    bq = nc.declare_dram_parameter("bq", [128, 2], F32, isOutput=False)
    bk = nc.declare_dram_parameter("bk", [128, 2], F32, isOutput=False)
    bv = nc.declare_dram_parameter("bv", [1, DQC], F32R, isOutput=False)
    y = nc.declare_dram_parameter("y", [DQC, S], F32, isOutput=True)

    with TileContext(nc) as tc:
        with (
            tc.tile_pool(name="const", bufs=1) as cpool,
            tc.tile_pool(name="attn", bufs=16) as apool,
            tc.tile_pool(name="work", bufs=2) as wpool,
            tc.tile_pool(name="mm_ps", bufs=2, space="PSUM") as mmps,
            tc.tile_pool(name="av_ps", bufs=1, space="PSUM") as avps,
        ):
            # ---- input loads (weights/biases first: the first projection
            # matmuls need wq[0]/xts[0], not the whole xT) -------------------
            wsb = {}
            for name, dram in (("q", wq), ("k", wk), ("v", wv)):
                t = cpool.tile([128, NKT * DQC], BF16, tag=f"w{name}", name=f"w{name}")
                nc.sync.dma_start(
                    out=t[:, :],
                    in_=dram[:, :].rearrange("(kt p) c -> p kt c", p=128),
                )
                wsb[name] = t[:, :].rearrange("p (kt c) -> p kt c", kt=NKT)

            bq_sb = cpool.tile([128, 2], F32, tag="bq")
            nc.sync.dma_start(out=bq_sb[:, :], in_=bq[:, :])
            bk_sb = cpool.tile([128, 2], F32, tag="bk")
            nc.sync.dma_start(out=bk_sb[:, :], in_=bk[:, :])
            bv_sb = cpool.tile([1, DQC], F32R, tag="bv")
            nc.sync.dma_start(out=bv_sb[:, :], in_=bv[:, :])
            ones_row = cpool.tile([1, 128], F32R, tag="ones_row")
            nc.vector.memset(
                ones_row[:, :].bitcast(mybir.dt.uint32), 0x3F800000
            )
            # bias-v broadcast to all partitions via the tensor engine
            bvb_ps = mmps.tile([128, DQC], F32, tag="mm", name="bvb_ps")
            nc.tensor.matmul(
                out=bvb_ps[:, :], lhsT=ones_row[:, :], rhs=bv_sb[:, :],
                start=True, stop=True,
            )
            bvb = cpool.tile([128, DQC], F32, tag="bvb")
            nc.vector.tensor_copy(bvb[:, :], bvb_ps[:, :])

            xts = []
            for kt in range(NKT):
                t = cpool.tile([128, S], BF16, tag=f"xt{kt}", name=f"xt{kt}")
                nc.sync.dma_start(out=t[:, :], in_=xT[kt * 128:(kt + 1) * 128, :])
                xts.append(t)

            # V' tile: [128 (s-chunk partitions), 16 s-chunks x (4 heads x 65)]
            vp = cpool.tile([128, NKC * HPC * VROW], BF16, tag="vp")
            vp4 = vp[:, :].rearrange(
                "p (sc h c) -> p sc h c", sc=NKC, h=HPC, c=VROW
            )
            nc.vector.memset(vp4[:, :, :, DH:VROW], 1.0)

            # ---- qkv^T projection -------------------------------------------
            # Q^T, K^T chunk tiles: [128 (channel), 2048 (s)] x 2 chunks each,
            # emitted m-chunk-major so heads 0/1 are ready early.
            qth, kth = [None] * HPC, [None] * HPC
            qtc, ktc = [None, None], [None, None]
            for m in range(2):
                for which, chunks, wt, bias in (
                    ("q", qtc, wsb["q"], bq_sb),
                    ("k", ktc, wsb["k"], bk_sb),
                ):
                    chunk = cpool.tile([128, S], BF16, tag=f"{which}tc{m}", name=f"{which}tc{m}")
                    chunks[m] = chunk
                    for qb in range(NQB):
                        ps = mmps.tile([128, 512], F32, tag="mm", name=f"qkps_{which}{m}_{qb}")
                        for kt in range(NKT):
                            nc.tensor.matmul(
                                out=ps[:, :],
                                lhsT=wt[:, kt, m * 128:(m + 1) * 128],
                                rhs=xts[kt][:, qb * 512:(qb + 1) * 512],
                                start=(kt == 0),
                                stop=(kt == NKT - 1),
                            )
                        nc.vector.tensor_scalar(
                            out=chunk[:, qb * 512:(qb + 1) * 512],
                            in0=ps[:, :],
                            scalar1=bias[:, m:m + 1],
                            scalar2=None,
                            op0=ADD,
                        )
                # even head: direct slice; odd head: SBUF->SBUF DMA down to
                # partition base 0 (base-64 matmul APs would engage the
                # row-tiling encoding)
                qth[2 * m] = qtc[m][0:64, :]
                kth[2 * m] = ktc[m][0:64, :]
                h = 2 * m + 1
                qo = cpool.tile([64, S], BF16, tag=f"qto{h}", name=f"qto{h}")
                nc.sync.dma_start(out=qo[:, :], in_=qtc[m][64:128, :])
                qth[h] = qo[:, :]
                ko = cpool.tile([64, S], BF16, tag=f"kto{h}", name=f"kto{h}")
                nc.sync.dma_start(out=ko[:, :], in_=ktc[m][64:128, :])
                kth[h] = ko[:, :]

            # V (+bias) in [s, channel] layout, strided into V' head groups.
            for sc in range(NKC):
                ps = mmps.tile([128, DQC], F32, tag="mm", name=f"vps_{sc}")
                for kt in range(NKT):
                    nc.tensor.matmul(
                        out=ps[:, :],
                        lhsT=xts[kt][:, sc * 128:(sc + 1) * 128],
                        rhs=wsb["v"][:, kt, :],
                        start=(kt == 0),
                        stop=(kt == NKT - 1),
                    )
                nc.vector.tensor_tensor(
                    out=vp4[:, sc, :, 0:DH],
                    in0=ps[:, :].rearrange("p (h c) -> p h c", h=HPC),
                    in1=bvb[:, :].rearrange("p (h c) -> p h c", h=HPC),
                    op=ADD,
                )

            # ---- attention ---------------------------------------------------
            attn = [[None] * NKC for _ in range(HPC)]
            av_out = [None] * HPC

            def emit_scores(h, kc):
                at = apool.tile([128, S], BF16, tag="attn", name=f"attn{h}_{kc}")
                attn[h][kc] = at
                for half in range(2):
                    ps = mmps.tile([128, 1024], F32, tag="mm", name=f"sps_{h}_{kc}_{half}")
                    for j in range(2):
                        qb = 2 * half + j
                        nc.tensor.matmul(
                            out=ps[:, j * 512:(j + 1) * 512],
                            lhsT=kth[h][:, kc * 128:(kc + 1) * 128],
                            rhs=qth[h][:, qb * 512:(qb + 1) * 512],
                            start=True,
                            stop=True,
                        )
                    nc.scalar.activation(
                        out=at[:, half * 1024:(half + 1) * 1024],
                        in_=ps[:, :],
                        func=mybir.ActivationFunctionType.Exp,
                        scale=1.0 / np.sqrt(DH),
                    )

            def emit_av(h, kc):
                if kc == 0:
                    av_out[h] = avps.tile([VROW, S], F32, tag="av", name=f"av{h}")
                ps = av_out[h]
                for qb in range(NQB):
                    nc.tensor.matmul(
                        out=ps[:, qb * 512:(qb + 1) * 512],
                        lhsT=vp4[:, kc, h, :],
                        rhs=attn[h][kc][:, qb * 512:(qb + 1) * 512],
                        start=(kc == 0),
                        stop=(kc == NKC - 1),
                    )

            def emit_norm(h):
                ps = av_out[h]
                # Copy PSUM -> SBUF first so the single av_ps slot frees as
                # early as possible (the next head's AV matmuls wait on it).
                ot = wpool.tile([VROW, S], F32, tag="out", name=f"ot{h}")
                nc.vector.tensor_copy(ot[:, :], ps[:, :])
                # Reciprocal of the denominators (ones column of V' summed
                # into row DH). A [1, S] reciprocal runs on one DVE lane at
                # ~7.5 cyc/elem (~13us); scatter the row across 64 partitions
                # first so it takes S/DH elems per lane instead.
                rsc = wpool.tile([DH, S // DH], F32, tag="rsc", name=f"rsc{h}")
                nc.sync.dma_start(out=rsc[:, :], in_=ot[DH:VROW, :])
                rscr = wpool.tile([DH, S // DH], F32R, tag="rscr", name=f"rscr{h}")
                with nc.allow_low_precision(reason="f32r feed for broadcast matmul"):
                    nc.vector.reciprocal(out=rscr[:, :], in_=rsc[:, :])
                rrow = wpool.tile([1, S], F32R, tag="rrow", name=f"rrow{h}")
                nc.sync.dma_start(out=rrow[:, :], in_=rscr[:, :])
                # Broadcast the reciprocal row to 64 partitions with the
                # tensor engine; the output reuses the just-freed av_ps slot
                # so the scores psum rotation never waits on this chain.
                rbp = avps.tile([DH, S], F32, tag="av", name=f"rb{h}")
                for qb in range(NQB):
                    nc.tensor.matmul(
                        out=rbp[:, qb * 512:(qb + 1) * 512],
                        lhsT=ones_row[:, 0:DH],
                        rhs=rrow[:, qb * 512:(qb + 1) * 512],
                        start=True,
                        stop=True,
                    )
                nc.vector.tensor_tensor(
                    out=ot[0:DH, :], in0=ot[0:DH, :], in1=rbp[:, :], op=MULT
                )
                nc.sync.dma_start(out=y[h * DH:(h + 1) * DH, :], in_=ot[0:DH, :])

            for h in range(HPC):
                for kc in range(NKC):
                    emit_scores(h, kc)
                    if h > 0:
                        emit_av(h - 1, kc)
                if h > 0:
                    emit_norm(h - 1)
            for kc in range(NKC):
                emit_av(HPC - 1, kc)
            emit_norm(HPC - 1)

    if split_waits:
        _split_excess_waits(nc)
    return nc


_NC = None


def _get_nc() -> bass.Bass:
    global _NC
    if _NC is None:
        _NC = _build_nc()
    return _NC


def make_in_maps(x, W_qkv, b_qkv):
    x = np.asarray(x, dtype=np.float32)
    W = np.asarray(W_qkv, dtype=np.float32)
    b = np.asarray(b_qkv, dtype=np.float32)
    in_maps = []
    for c in range(8):
        bi, g = divmod(c, 4)
        cols = slice(g * DQC, (g + 1) * DQC)
        xT = np.ascontiguousarray(x[bi].T).astype(NPBF16)
        m = {
            "xT": xT,
            "wq": np.ascontiguousarray(W[:, 0:D][:, cols]).astype(NPBF16),
            "wk": np.ascontiguousarray(W[:, D:2 * D][:, cols]).astype(NPBF16),
            "wv": np.ascontiguousarray(W[:, 2 * D:3 * D][:, cols]).astype(NPBF16),
            "bq": np.ascontiguousarray(b[0:D][cols].reshape(2, 128).T),
            "bk": np.ascontiguousarray(b[D:2 * D][cols].reshape(2, 128).T),
            "bv": b[2 * D:3 * D][cols].reshape(1, DQC).copy(),
        }
        in_maps.append(m)
    return in_maps


def gather_out(results):
    out = np.zeros((2, S, D), np.float32)
    for c in range(8):
        bi, g = divmod(c, 4)
        out[bi, :, g * DQC:(g + 1) * DQC] = np.asarray(
            results[c]["y"], np.float32
        ).T
    return out


def kernel(x, W_qkv, b_qkv):
    nc = _get_nc()
    in_maps = make_in_maps(x, W_qkv, b_qkv)
    trace = bool(int(os.environ.get("BASS_KERNEL_TRACE", "0")))
    res = run_bass_kernel_spmd(nc, in_maps, list(range(8)), trace=trace)
    if trace:
        kernel.last_result = res
    return gather_out(res.results)


# revision 23
# speedup vs baseline: 1.3880x; 1.0012x over previous
"""Multi-head attention (B=2, S=2048, D=1024, H=16) on 8 trn2 NeuronCores.

Sharding: core c -> batch b = c // 4, head-group g = c % 4 (4 heads/core).
Each core computes, for its batch and its 4 heads:
    qkv^T projection -> per-head scores^T = K Q^T / 8 -> exp -> AV with an
    appended ones-column on V (gives softmax denominators for free) ->
    normalize -> out^T [256, 2048].
Host transposes x per batch (so the contraction dim lands on partitions),
casts matmul operands to bf16, and transposes/scatters the per-core outputs
back into the full [2, 2048, 1024] f32 result.

All matmuls run in the standard 128x128 PE mode (operands are arranged so
every lhsT/rhs AP starts at partition 0 or is 128 partitions tall - avoids
the row-tiling encoding and its mode-switch drains).
"""

import os

import numpy as np
import ml_dtypes

import concourse.bass as bass
import concourse.mybir as mybir
from concourse.bass_utils import run_bass_kernel_spmd
from concourse.tile import TileContext
from concourse.vector_clock import ScopedClock

S = 2048          # sequence length
D = 1024          # embed dim
HPC = 4           # heads per core
DH = 64           # head dim
DQC = HPC * DH    # q/k/v channels per core (256)
NKT = D // 128    # contraction tiles for the projection (8)
NKC = S // 128    # key-position chunks (16)
NQB = S // 512    # query blocks (4)
VROW = DH + 1     # V' columns per head (64 values + ones column)

BF16 = mybir.dt.bfloat16
F32 = mybir.dt.float32
F32R = mybir.dt.float32r
NPBF16 = ml_dtypes.bfloat16
ADD = mybir.AluOpType.add
MULT = mybir.AluOpType.mult


def _split_excess_waits(nc: bass.Bass, cap: int = 1) -> None:
    """The walrus build in this container supports at most one sync-wait
    command per ISA instruction; Tile attaches one wait per producer. Move
    excess waits onto same-engine NOPs inserted just before the offender
    (engine queues are FIFO, so the NOP waits gate everything behind them)."""
    n = 0
    for f in nc.m.functions:
        for blk in f.blocks:
            out = []
            for inst in blk.instructions:
                si = inst.sync_info
                waits = list(si.on_wait) if si is not None and si.on_wait else []
                if len(waits) > cap:
                    for w in waits[:-cap]:
                        n += 1
                        nop = mybir.InstNoOp(
                            name=f"{inst.name}-ws{n}", ins=[], outs=[]
                        )
                        nop.engine = inst.engine
                        nop.sync_info = mybir.SyncInfo(on_wait=[w], on_update=[])
                        out.append(nop)
                    inst.sync_info = mybir.SyncInfo(
                        on_wait=waits[-cap:],
                        on_update=list(si.on_update) if si.on_update else [],
                    )
                out.append(inst)
            blk.instructions = out


def _build_nc(split_waits: bool = True) -> bass.Bass:
    nc = bass.Bass()
    xT = nc.declare_dram_parameter("xT", [D, S], BF16, isOutput=False)
    wq = nc.declare_dram_parameter("wq", [D, DQC], BF16, isOutput=False)
    wk = nc.declare_dram_parameter("wk", [D, DQC], BF16, isOutput=False)
    wv = nc.declare_dram_parameter("wv", [D, DQC], BF16, isOutput=False)
    bq = nc.declare_dram_parameter("bq", [128, 2], F32, isOutput=False)
    bk = nc.declare_dram_parameter("bk", [128, 2], F32, isOutput=False)
    bv = nc.declare_dram_parameter("bv", [1, DQC], F32R, isOutput=False)
    y = nc.declare_dram_parameter("y", [DQC, S], F32, isOutput=True)

    with TileContext(nc) as tc:
        with (
            tc.tile_pool(name="const", bufs=1) as cpool,
            tc.tile_pool(name="attn", bufs=16) as apool,
            tc.tile_pool(name="work", bufs=2) as wpool,
            tc.tile_pool(name="mm_ps", bufs=2, space="PSUM") as mmps,
            tc.tile_pool(name="av_ps", bufs=1, space="PSUM") as avps,
        ):
            # ---- input loads (weights/biases first: the first projection
            # matmuls need wq[0]/xts[0], not the whole xT) -------------------
            wsb = {}
            for name, dram in (("q", wq), ("k", wk), ("v", wv)):
                t = cpool.tile([128, NKT * DQC], BF16, tag=f"w{name}", name=f"w{name}")
                nc.sync.dma_start(
                    out=t[:, :],
                    in_=dram[:, :].rearrange("(kt p) c -> p kt c", p=128),
                )
                wsb[name] = t[:, :].rearrange("p (kt c) -> p kt c", kt=NKT)

            bq_sb = cpool.tile([128, 2], F32, tag="bq")
            nc.sync.dma_start(out=bq_sb[:, :], in_=bq[:, :])
            bk_sb = cpool.tile([128, 2], F32, tag="bk")
            nc.sync.dma_start(out=bk_sb[:, :], in_=bk[:, :])
            bv_sb = cpool.tile([1, DQC], F32R, tag="bv")
            nc.sync.dma_start(out=bv_sb[:, :], in_=bv[:, :])
            ones_row = cpool.tile([1, 128], F32R, tag="ones_row")
            nc.vector.memset(
                ones_row[:, :].bitcast(mybir.dt.uint32), 0x3F800000
            )
            # bias-v broadcast to all partitions via the tensor engine
            bvb_ps = mmps.tile([128, DQC], F32, tag="mm", name="bvb_ps")
            nc.tensor.matmul(
                out=bvb_ps[:, :], lhsT=ones_row[:, :], rhs=bv_sb[:, :],
                start=True, stop=True,
            )
            bvb = cpool.tile([128, DQC], F32, tag="bvb")
            nc.vector.tensor_copy(bvb[:, :], bvb_ps[:, :])

            xts = []
            for kt in range(NKT):
                t = cpool.tile([128, S], BF16, tag=f"xt{kt}", name=f"xt{kt}")
                nc.sync.dma_start(out=t[:, :], in_=xT[kt * 128:(kt + 1) * 128, :])
                xts.append(t)

            # V' tile: [128 (s-chunk partitions), 16 s-chunks x (4 heads x 65)]
            vp = cpool.tile([128, NKC * HPC * VROW], BF16, tag="vp")
            vp4 = vp[:, :].rearrange(
                "p (sc h c) -> p sc h c", sc=NKC, h=HPC, c=VROW
            )
            nc.vector.memset(vp4[:, :, :, DH:VROW], 1.0)

            # ---- qkv^T projection -------------------------------------------
            # Q^T, K^T chunk tiles: [128 (channel), 2048 (s)] x 2 chunks each.
            qth, kth = [None] * HPC, [None] * HPC
            qtc, ktc = [None, None], [None, None]

            def emit_qk_chunk(m):
                for which, chunks, wt, bias in (
                    ("q", qtc, wsb["q"], bq_sb),
                    ("k", ktc, wsb["k"], bk_sb),
                ):
                    chunk = cpool.tile([128, S], BF16, tag=f"{which}tc{m}", name=f"{which}tc{m}")
                    chunks[m] = chunk
                    for qb in range(NQB):
                        ps = mmps.tile([128, 512], F32, tag="mm", name=f"qkps_{which}{m}_{qb}")
                        for kt in range(NKT):
                            nc.tensor.matmul(
                                out=ps[:, :],
                                lhsT=wt[:, kt, m * 128:(m + 1) * 128],
                                rhs=xts[kt][:, qb * 512:(qb + 1) * 512],
                                start=(kt == 0),
                                stop=(kt == NKT - 1),
                            )
                        nc.vector.tensor_scalar(
                            out=chunk[:, qb * 512:(qb + 1) * 512],
                            in0=ps[:, :],
                            scalar1=bias[:, m:m + 1],
                            scalar2=None,
                            op0=ADD,
                        )
                # even head: direct slice; odd head: SBUF->SBUF DMA down to
                # partition base 0 (base-64 matmul APs would engage the
                # row-tiling encoding)
                qth[2 * m] = qtc[m][0:64, :]
                kth[2 * m] = ktc[m][0:64, :]
                h = 2 * m + 1
                qo = cpool.tile([64, S], BF16, tag=f"qto{h}", name=f"qto{h}")
                nc.sync.dma_start(out=qo[:, :], in_=qtc[m][64:128, :])
                qth[h] = qo[:, :]
                ko = cpool.tile([64, S], BF16, tag=f"kto{h}", name=f"kto{h}")
                nc.sync.dma_start(out=ko[:, :], in_=ktc[m][64:128, :])
                kth[h] = ko[:, :]

            def emit_v_chunk(sc):
                # V (+bias) in [s, channel] layout, strided into V' groups.
                ps = mmps.tile([128, DQC], F32, tag="mm", name=f"vps_{sc}")
                for kt in range(NKT):
                    nc.tensor.matmul(
                        out=ps[:, :],
                        lhsT=xts[kt][:, sc * 128:(sc + 1) * 128],
                        rhs=wsb["v"][:, kt, :],
                        start=(kt == 0),
                        stop=(kt == NKT - 1),
                    )
                nc.vector.tensor_tensor(
                    out=vp4[:, sc, :, 0:DH],
                    in0=ps[:, :].rearrange("p (h c) -> p h c", h=HPC),
                    in1=bvb[:, :].rearrange("p (h c) -> p h c", h=HPC),
                    op=ADD,
                )

            # ---- attention ---------------------------------------------------
            attn = [[None] * NKC for _ in range(HPC)]
            av_out = [None] * HPC

            def emit_scores(h, kc):
                at = apool.tile([128, S], BF16, tag="attn", name=f"attn{h}_{kc}")
                attn[h][kc] = at
                for half in range(2):
                    ps = mmps.tile([128, 1024], F32, tag="mm", name=f"sps_{h}_{kc}_{half}")
                    for j in range(2):
                        qb = 2 * half + j
                        nc.tensor.matmul(
                            out=ps[:, j * 512:(j + 1) * 512],
                            lhsT=kth[h][:, kc * 128:(kc + 1) * 128],
                            rhs=qth[h][:, qb * 512:(qb + 1) * 512],
                            start=True,
                            stop=True,
                        )
                    nc.scalar.activation(
                        out=at[:, half * 1024:(half + 1) * 1024],
                        in_=ps[:, :],
                        func=mybir.ActivationFunctionType.Exp,
                        scale=1.0 / np.sqrt(DH),
                    )

            def emit_av(h, kc):
                if kc == 0:
                    av_out[h] = avps.tile([VROW, S], F32, tag="av", name=f"av{h}")
                ps = av_out[h]
                for qb in range(NQB):
                    nc.tensor.matmul(
                        out=ps[:, qb * 512:(qb + 1) * 512],
                        lhsT=vp4[:, kc, h, :],
                        rhs=attn[h][kc][:, qb * 512:(qb + 1) * 512],
                        start=(kc == 0),
                        stop=(kc == NKC - 1),
                    )

            def emit_norm(h):
                ps = av_out[h]
                # Copy PSUM -> SBUF first so the single av_ps slot frees as
                # early as possible (the next head's AV matmuls wait on it).
                ot = wpool.tile([VROW, S], F32, tag="out", name=f"ot{h}")
                nc.vector.tensor_copy(ot[:, :], ps[:, :])
                # Reciprocal of the denominators (ones column of V' summed
                # into row DH). A [1, S] reciprocal runs on one DVE lane at
                # ~7.5 cyc/elem (~13us); scatter the row across 64 partitions
                # first so it takes S/DH elems per lane instead.
                rsc = wpool.tile([DH, S // DH], F32, tag="rsc", name=f"rsc{h}")
                nc.sync.dma_start(out=rsc[:, :], in_=ot[DH:VROW, :])
                rscr = wpool.tile([DH, S // DH], F32R, tag="rscr", name=f"rscr{h}")
                with nc.allow_low_precision(reason="f32r feed for broadcast matmul"):
                    nc.vector.reciprocal(out=rscr[:, :], in_=rsc[:, :])
                rrow = wpool.tile([1, S], F32R, tag="rrow", name=f"rrow{h}")
                nc.sync.dma_start(out=rrow[:, :], in_=rscr[:, :])
                # Broadcast the reciprocal row to 64 partitions with the
                # tensor engine; the output reuses the just-freed av_ps slot
                # so the scores psum rotation never waits on this chain.
                rbp = avps.tile([DH, S], F32, tag="av", name=f"rb{h}")
                for qb in range(NQB):
                    nc.tensor.matmul(
                        out=rbp[:, qb * 512:(qb + 1) * 512],
                        lhsT=ones_row[:, 0:DH],
                        rhs=rrow[:, qb * 512:(qb + 1) * 512],
                        start=True,
                        stop=True,
                    )
                nc.vector.tensor_tensor(
                    out=ot[0:DH, :], in0=ot[0:DH, :], in1=rbp[:, :], op=MULT
                )
                nc.sync.dma_start(out=y[h * DH:(h + 1) * DH, :], in_=ot[0:DH, :])

            emit_qk_chunk(0)
            rest = [lambda m=1: emit_qk_chunk(1)] + [
                lambda sc=sc: emit_v_chunk(sc) for sc in range(NKC)
            ]
            for kc in range(NKC):
                emit_scores(0, kc)
                if kc < 2 and rest:
                    rest.pop(0)()
                elif rest:
                    rest.pop(0)()
            while rest:
                rest.pop(0)()

            for h in range(1, HPC):
                for kc in range(NKC):
                    emit_scores(h, kc)
                    emit_av(h - 1, kc)
                emit_norm(h - 1)
            for kc in range(NKC):
                emit_av(HPC - 1, kc)
            emit_norm(HPC - 1)

    if split_waits:
        _split_excess_waits(nc)
    return nc


_NC = None


def _get_nc() -> bass.Bass:
    global _NC
    if _NC is None:
        _NC = _build_nc()
    return _NC


def make_in_maps(x, W_qkv, b_qkv):
    x = np.asarray(x, dtype=np.float32)
    W = np.asarray(W_qkv, dtype=np.float32)
    b = np.asarray(b_qkv, dtype=np.float32)
    in_maps = []
    for c in range(8):
        bi, g = divmod(c, 4)
        cols = slice(g * DQC, (g + 1) * DQC)
        xT = np.ascontiguousarray(x[bi].T).astype(NPBF16)
        m = {
            "xT": xT,
            "wq": np.ascontiguousarray(W[:, 0:D][:, cols]).astype(NPBF16),
            "wk": np.ascontiguousarray(W[:, D:2 * D][:, cols]).astype(NPBF16),
            "wv": np.ascontiguousarray(W[:, 2 * D:3 * D][:, cols]).astype(NPBF16),
            "bq": np.ascontiguousarray(b[0:D][cols].reshape(2, 128).T),
            "bk": np.ascontiguousarray(b[D:2 * D][cols].reshape(2, 128).T),
            "bv": b[2 * D:3 * D][cols].reshape(1, DQC).copy(),
        }
        in_maps.append(m)
    return in_maps


def gather_out(results):
    out = np.zeros((2, S, D), np.float32)
    for c in range(8):
        bi, g = divmod(c, 4)
        out[bi, :, g * DQC:(g + 1) * DQC] = np.asarray(
            results[c]["y"], np.float32
        ).T
    return out


def kernel(x, W_qkv, b_qkv):
    nc = _get_nc()
    in_maps = make_in_maps(x, W_qkv, b_qkv)
    trace = bool(int(os.environ.get("BASS_KERNEL_TRACE", "0")))
    res = run_bass_kernel_spmd(nc, in_maps, list(range(8)), trace=trace)
    if trace:
        kernel.last_result = res
    return gather_out(res.results)


# revision 24
# speedup vs baseline: 1.4115x; 1.0169x over previous
"""Multi-head attention (B=2, S=2048, D=1024, H=16) on 8 trn2 NeuronCores.

Sharding: core c -> batch b = c // 4, head-group g = c % 4 (4 heads/core).
Each core computes, for its batch and its 4 heads:
    qkv^T projection -> per-head scores^T = K Q^T / 8 -> exp -> AV with an
    appended ones-column on V (gives softmax denominators for free) ->
    normalize -> out^T [256, 2048].
Host transposes x per batch (so the contraction dim lands on partitions),
casts matmul operands to bf16, and transposes/scatters the per-core outputs
back into the full [2, 2048, 1024] f32 result.

All matmuls run in the standard 128x128 PE mode (operands are arranged so
every lhsT/rhs AP starts at partition 0 or is 128 partitions tall - avoids
the row-tiling encoding and its mode-switch drains).
"""

import os

import numpy as np
import ml_dtypes

import concourse.bass as bass
import concourse.mybir as mybir
from concourse.bass_utils import run_bass_kernel_spmd
from concourse.tile import TileContext
from concourse.vector_clock import ScopedClock

S = 2048          # sequence length
D = 1024          # embed dim
HPC = 4           # heads per core
DH = 64           # head dim
DQC = HPC * DH    # q/k/v channels per core (256)
NKT = D // 128    # contraction tiles for the projection (8)
NKC = S // 128    # key-position chunks (16)
NQB = S // 512    # query blocks (4)
VROW = DH + 1     # V' columns per head (64 values + ones column)

BF16 = mybir.dt.bfloat16
F32 = mybir.dt.float32
F32R = mybir.dt.float32r
NPBF16 = ml_dtypes.bfloat16
ADD = mybir.AluOpType.add
MULT = mybir.AluOpType.mult


def _split_excess_waits(nc: bass.Bass, cap: int = 1) -> None:
    """The walrus build in this container supports at most one sync-wait
    command per ISA instruction; Tile attaches one wait per producer. Move
    excess waits onto same-engine NOPs inserted just before the offender
    (engine queues are FIFO, so the NOP waits gate everything behind them)."""
    n = 0
    for f in nc.m.functions:
        for blk in f.blocks:
            out = []
            for inst in blk.instructions:
                si = inst.sync_info
                waits = list(si.on_wait) if si is not None and si.on_wait else []
                if len(waits) > cap:
                    for w in waits[:-cap]:
                        n += 1
                        nop = mybir.InstNoOp(
                            name=f"{inst.name}-ws{n}", ins=[], outs=[]
                        )
                        nop.engine = inst.engine
                        nop.sync_info = mybir.SyncInfo(on_wait=[w], on_update=[])
                        out.append(nop)
                    inst.sync_info = mybir.SyncInfo(
                        on_wait=waits[-cap:],
                        on_update=list(si.on_update) if si.on_update else [],
                    )
                out.append(inst)
            blk.instructions = out


def _build_nc(split_waits: bool = True) -> bass.Bass:
    nc = bass.Bass()
    xT = nc.declare_dram_parameter("xT", [D, S], BF16, isOutput=False)
    wq = nc.declare_dram_parameter("wq", [128, NKT * DQC], BF16, isOutput=False)
    wk = nc.declare_dram_parameter("wk", [128, NKT * DQC], BF16, isOutput=False)
    wv = nc.declare_dram_parameter("wv", [128, NKT * DQC], BF16, isOutput=False)
    bq = nc.declare_dram_parameter("bq", [128, 2], F32, isOutput=False)
    bk = nc.declare_dram_parameter("bk", [128, 2], F32, isOutput=False)
    bv = nc.declare_dram_parameter("bv", [1, DQC], F32R, isOutput=False)
    y = nc.declare_dram_parameter("y", [DQC, S], F32, isOutput=True)

    with TileContext(nc) as tc:
        with (
            tc.tile_pool(name="const", bufs=1) as cpool,
            tc.tile_pool(name="attn", bufs=16) as apool,
            tc.tile_pool(name="work", bufs=2) as wpool,
            tc.tile_pool(name="mm_ps", bufs=2, space="PSUM") as mmps,
            tc.tile_pool(name="av_ps", bufs=1, space="PSUM") as avps,
        ):
            # ---- input loads (weights/biases first: the first projection
            # matmuls need wq[0]/xts[0], not the whole xT) -------------------
            wsb = {}
            for name, dram in (("q", wq), ("k", wk), ("v", wv)):
                t = cpool.tile([128, NKT * DQC], BF16, tag=f"w{name}", name=f"w{name}")
                nc.sync.dma_start(out=t[:, :], in_=dram[:, :])
                wsb[name] = t[:, :].rearrange("p (kt c) -> p kt c", kt=NKT)

            bq_sb = cpool.tile([128, 2], F32, tag="bq")
            nc.sync.dma_start(out=bq_sb[:, :], in_=bq[:, :])
            bk_sb = cpool.tile([128, 2], F32, tag="bk")
            nc.sync.dma_start(out=bk_sb[:, :], in_=bk[:, :])
            bv_sb = cpool.tile([1, DQC], F32R, tag="bv")
            nc.sync.dma_start(out=bv_sb[:, :], in_=bv[:, :])
            ones_row = cpool.tile([1, 128], F32R, tag="ones_row")
            nc.vector.memset(
                ones_row[:, :].bitcast(mybir.dt.uint32), 0x3F800000
            )
            # bias-v broadcast to all partitions via the tensor engine
            bvb_ps = mmps.tile([128, DQC], F32, tag="mm", name="bvb_ps")
            nc.tensor.matmul(
                out=bvb_ps[:, :], lhsT=ones_row[:, :], rhs=bv_sb[:, :],
                start=True, stop=True,
            )
            bvb = cpool.tile([128, DQC], F32, tag="bvb")
            nc.vector.tensor_copy(bvb[:, :], bvb_ps[:, :])

            xts = []
            for kt in range(NKT):
                t = cpool.tile([128, S], BF16, tag=f"xt{kt}", name=f"xt{kt}")
                nc.sync.dma_start(out=t[:, :], in_=xT[kt * 128:(kt + 1) * 128, :])
                xts.append(t)

            # V' tile: [128 (s-chunk partitions), 16 s-chunks x (4 heads x 65)]
            vp = cpool.tile([128, NKC * HPC * VROW], BF16, tag="vp")
            vp4 = vp[:, :].rearrange(
                "p (sc h c) -> p sc h c", sc=NKC, h=HPC, c=VROW
            )
            nc.vector.memset(vp4[:, :, :, DH:VROW], 1.0)

            # ---- qkv^T projection -------------------------------------------
            # Q^T, K^T chunk tiles: [128 (channel), 2048 (s)] x 2 chunks each.
            qth, kth = [None] * HPC, [None] * HPC
            qtc, ktc = [None, None], [None, None]

            def emit_qk_chunk(m):
                for which, chunks, wt, bias in (
                    ("q", qtc, wsb["q"], bq_sb),
                    ("k", ktc, wsb["k"], bk_sb),
                ):
                    chunk = cpool.tile([128, S], BF16, tag=f"{which}tc{m}", name=f"{which}tc{m}")
                    chunks[m] = chunk
                    for qb in range(NQB):
                        ps = mmps.tile([128, 512], F32, tag="mm", name=f"qkps_{which}{m}_{qb}")
                        for kt in range(NKT):
                            nc.tensor.matmul(
                                out=ps[:, :],
                                lhsT=wt[:, kt, m * 128:(m + 1) * 128],
                                rhs=xts[kt][:, qb * 512:(qb + 1) * 512],
                                start=(kt == 0),
                                stop=(kt == NKT - 1),
                            )
                        nc.vector.tensor_scalar(
                            out=chunk[:, qb * 512:(qb + 1) * 512],
                            in0=ps[:, :],
                            scalar1=bias[:, m:m + 1],
                            scalar2=None,
                            op0=ADD,
                        )
                # even head: direct slice; odd head: SBUF->SBUF DMA down to
                # partition base 0 (base-64 matmul APs would engage the
                # row-tiling encoding)
                qth[2 * m] = qtc[m][0:64, :]
                kth[2 * m] = ktc[m][0:64, :]
                h = 2 * m + 1
                qo = cpool.tile([64, S], BF16, tag=f"qto{h}", name=f"qto{h}")
                nc.sync.dma_start(out=qo[:, :], in_=qtc[m][64:128, :])
                qth[h] = qo[:, :]
                ko = cpool.tile([64, S], BF16, tag=f"kto{h}", name=f"kto{h}")
                nc.sync.dma_start(out=ko[:, :], in_=ktc[m][64:128, :])
                kth[h] = ko[:, :]

            def emit_v_chunk(sc):
                # V (+bias) in [s, channel] layout, strided into V' groups.
                ps = mmps.tile([128, DQC], F32, tag="mm", name=f"vps_{sc}")
                for kt in range(NKT):
                    nc.tensor.matmul(
                        out=ps[:, :],
                        lhsT=xts[kt][:, sc * 128:(sc + 1) * 128],
                        rhs=wsb["v"][:, kt, :],
                        start=(kt == 0),
                        stop=(kt == NKT - 1),
                    )
                nc.vector.tensor_tensor(
                    out=vp4[:, sc, :, 0:DH],
                    in0=ps[:, :].rearrange("p (h c) -> p h c", h=HPC),
                    in1=bvb[:, :].rearrange("p (h c) -> p h c", h=HPC),
                    op=ADD,
                )

            # ---- attention ---------------------------------------------------
            attn = [[None] * NKC for _ in range(HPC)]
            av_out = [None] * HPC

            def emit_scores(h, kc):
                at = apool.tile([128, S], BF16, tag="attn", name=f"attn{h}_{kc}")
                attn[h][kc] = at
                for half in range(2):
                    ps = mmps.tile([128, 1024], F32, tag="mm", name=f"sps_{h}_{kc}_{half}")
                    for j in range(2):
                        qb = 2 * half + j
                        nc.tensor.matmul(
                            out=ps[:, j * 512:(j + 1) * 512],
                            lhsT=kth[h][:, kc * 128:(kc + 1) * 128],
                            rhs=qth[h][:, qb * 512:(qb + 1) * 512],
                            start=True,
                            stop=True,
                        )
                    nc.scalar.activation(
                        out=at[:, half * 1024:(half + 1) * 1024],
                        in_=ps[:, :],
                        func=mybir.ActivationFunctionType.Exp,
                        scale=1.0 / np.sqrt(DH),
                    )

            def emit_av(h, kc):
                if kc == 0:
                    av_out[h] = avps.tile([VROW, S], F32, tag="av", name=f"av{h}")
                ps = av_out[h]
                for qb in range(NQB):
                    nc.tensor.matmul(
                        out=ps[:, qb * 512:(qb + 1) * 512],
                        lhsT=vp4[:, kc, h, :],
                        rhs=attn[h][kc][:, qb * 512:(qb + 1) * 512],
                        start=(kc == 0),
                        stop=(kc == NKC - 1),
                    )

            def emit_norm(h):
                ps = av_out[h]
                # Copy PSUM -> SBUF first so the single av_ps slot frees as
                # early as possible (the next head's AV matmuls wait on it).
                ot = wpool.tile([VROW, S], F32, tag="out", name=f"ot{h}")
                nc.vector.tensor_copy(ot[:, :], ps[:, :])
                # Reciprocal of the denominators (ones column of V' summed
                # into row DH). A [1, S] reciprocal runs on one DVE lane at
                # ~7.5 cyc/elem (~13us); scatter the row across 64 partitions
                # first so it takes S/DH elems per lane instead.
                rsc = wpool.tile([DH, S // DH], F32, tag="rsc", name=f"rsc{h}")
                nc.sync.dma_start(out=rsc[:, :], in_=ot[DH:VROW, :])
                rscr = wpool.tile([DH, S // DH], F32R, tag="rscr", name=f"rscr{h}")
                with nc.allow_low_precision(reason="f32r feed for broadcast matmul"):
                    nc.vector.reciprocal(out=rscr[:, :], in_=rsc[:, :])
                rrow = wpool.tile([1, S], F32R, tag="rrow", name=f"rrow{h}")
                nc.sync.dma_start(out=rrow[:, :], in_=rscr[:, :])
                # Broadcast the reciprocal row to 64 partitions with the
                # tensor engine; the output reuses the just-freed av_ps slot
                # so the scores psum rotation never waits on this chain.
                rbp = avps.tile([DH, S], F32, tag="av", name=f"rb{h}")
                for qb in range(NQB):
                    nc.tensor.matmul(
                        out=rbp[:, qb * 512:(qb + 1) * 512],
                        lhsT=ones_row[:, 0:DH],
                        rhs=rrow[:, qb * 512:(qb + 1) * 512],
                        start=True,
                        stop=True,
                    )
                nc.vector.tensor_tensor(
                    out=ot[0:DH, :], in0=ot[0:DH, :], in1=rbp[:, :], op=MULT
                )
                nc.sync.dma_start(out=y[h * DH:(h + 1) * DH, :], in_=ot[0:DH, :])

            emit_qk_chunk(0)
            rest = [lambda m=1: emit_qk_chunk(1)] + [
                lambda sc=sc: emit_v_chunk(sc) for sc in range(NKC)
            ]
            for kc in range(NKC):
                emit_scores(0, kc)
                if kc < 2 and rest:
                    rest.pop(0)()
                elif rest:
                    rest.pop(0)()
            while rest:
                rest.pop(0)()

            LAG = 4
            for h in range(1, HPC):
                last = h == HPC - 1
                for kc in range(NKC):
                    emit_scores(h, kc)
                    emit_av(h - 1, kc)
                    if last and kc >= LAG:
                        emit_av(h, kc - LAG)
                emit_norm(h - 1)
            for kc in range(NKC - LAG, NKC):
                emit_av(HPC - 1, kc)
            emit_norm(HPC - 1)

    if split_waits:
        _split_excess_waits(nc)
    return nc


_NC = None


def _get_nc() -> bass.Bass:
    global _NC
    if _NC is None:
        _NC = _build_nc()
    return _NC


def make_in_maps(x, W_qkv, b_qkv):
    x = np.asarray(x, dtype=np.float32)
    W = np.asarray(W_qkv, dtype=np.float32)
    b = np.asarray(b_qkv, dtype=np.float32)
    in_maps = []
    for c in range(8):
        bi, g = divmod(c, 4)
        cols = slice(g * DQC, (g + 1) * DQC)
        xT = np.ascontiguousarray(x[bi].T).astype(NPBF16)
        def pack_w(block):
            # [1024, 256] -> [128 partitions, kt-major 8*256] contiguous
            w = block[:, cols].reshape(NKT, 128, DQC).transpose(1, 0, 2)
            return np.ascontiguousarray(w.reshape(128, NKT * DQC)).astype(NPBF16)

        m = {
            "xT": xT,
            "wq": pack_w(W[:, 0:D]),
            "wk": pack_w(W[:, D:2 * D]),
            "wv": pack_w(W[:, 2 * D:3 * D]),
            "bq": np.ascontiguousarray(b[0:D][cols].reshape(2, 128).T),
            "bk": np.ascontiguousarray(b[D:2 * D][cols].reshape(2, 128).T),
            "bv": b[2 * D:3 * D][cols].reshape(1, DQC).copy(),
        }
        in_maps.append(m)
    return in_maps


def gather_out(results):
    out = np.zeros((2, S, D), np.float32)
    for c in range(8):
        bi, g = divmod(c, 4)
        out[bi, :, g * DQC:(g + 1) * DQC] = np.asarray(
            results[c]["y"], np.float32
        ).T
    return out


def kernel(x, W_qkv, b_qkv):
    nc = _get_nc()
    in_maps = make_in_maps(x, W_qkv, b_qkv)
    trace = bool(int(os.environ.get("BASS_KERNEL_TRACE", "0")))
    res = run_bass_kernel_spmd(nc, in_maps, list(range(8)), trace=trace)
    if trace:
        kernel.last_result = res
    return gather_out(res.results)


# revision 25
# speedup vs baseline: 1.4146x; 1.0022x over previous
"""Multi-head attention (B=2, S=2048, D=1024, H=16) on 8 trn2 NeuronCores.

Sharding: core c -> batch b = c // 4, head-group g = c % 4 (4 heads/core).
Each core computes, for its batch and its 4 heads:
    qkv^T projection -> per-head scores^T = K Q^T / 8 -> exp -> AV with an
    appended ones-column on V (gives softmax denominators for free) ->
    normalize -> out^T [256, 2048].
Host transposes x per batch (so the contraction dim lands on partitions),
casts matmul operands to bf16, and transposes/scatters the per-core outputs
back into the full [2, 2048, 1024] f32 result.

All matmuls run in the standard 128x128 PE mode (operands are arranged so
every lhsT/rhs AP starts at partition 0 or is 128 partitions tall - avoids
the row-tiling encoding and its mode-switch drains).
"""

import os

import numpy as np
import ml_dtypes

import concourse.bass as bass
import concourse.mybir as mybir
from concourse.bass_utils import run_bass_kernel_spmd
from concourse.tile import TileContext
from concourse.vector_clock import ScopedClock

S = 2048          # sequence length
D = 1024          # embed dim
HPC = 4           # heads per core
DH = 64           # head dim
DQC = HPC * DH    # q/k/v channels per core (256)
NKT = D // 128    # contraction tiles for the projection (8)
NKC = S // 128    # key-position chunks (16)
NQB = S // 512    # query blocks (4)
VROW = DH + 1     # V' columns per head (64 values + ones column)

BF16 = mybir.dt.bfloat16
F32 = mybir.dt.float32
F32R = mybir.dt.float32r
NPBF16 = ml_dtypes.bfloat16
ADD = mybir.AluOpType.add
MULT = mybir.AluOpType.mult


def _split_excess_waits(nc: bass.Bass, cap: int = 1) -> None:
    """The walrus build in this container supports at most one sync-wait
    command per ISA instruction; Tile attaches one wait per producer. Move
    excess waits onto same-engine NOPs inserted just before the offender
    (engine queues are FIFO, so the NOP waits gate everything behind them)."""
    n = 0
    for f in nc.m.functions:
        for blk in f.blocks:
            out = []
            for inst in blk.instructions:
                si = inst.sync_info
                waits = list(si.on_wait) if si is not None and si.on_wait else []
                if len(waits) > cap:
                    for w in waits[:-cap]:
                        n += 1
                        nop = mybir.InstNoOp(
                            name=f"{inst.name}-ws{n}", ins=[], outs=[]
                        )
                        nop.engine = inst.engine
                        nop.sync_info = mybir.SyncInfo(on_wait=[w], on_update=[])
                        out.append(nop)
                    inst.sync_info = mybir.SyncInfo(
                        on_wait=waits[-cap:],
                        on_update=list(si.on_update) if si.on_update else [],
                    )
                out.append(inst)
            blk.instructions = out


def _build_nc(split_waits: bool = True) -> bass.Bass:
    nc = bass.Bass()
    xT = nc.declare_dram_parameter("xT", [D, S], BF16, isOutput=False)
    wq = nc.declare_dram_parameter("wq", [128, NKT * DQC], BF16, isOutput=False)
    wk = nc.declare_dram_parameter("wk", [128, NKT * DQC], BF16, isOutput=False)
    wv = nc.declare_dram_parameter("wv", [128, NKT * DQC], BF16, isOutput=False)
    bq = nc.declare_dram_parameter("bq", [128, 2], F32, isOutput=False)
    bk = nc.declare_dram_parameter("bk", [128, 2], F32, isOutput=False)
    bv = nc.declare_dram_parameter("bv", [1, DQC], F32R, isOutput=False)
    y = nc.declare_dram_parameter("y", [DQC, S], F32, isOutput=True)

    with TileContext(nc) as tc:
        with (
            tc.tile_pool(name="const", bufs=1) as cpool,
            tc.tile_pool(name="attn", bufs=16) as apool,
            tc.tile_pool(name="work", bufs=2) as wpool,
            tc.tile_pool(name="mm_ps", bufs=2, space="PSUM") as mmps,
            tc.tile_pool(name="av_ps", bufs=1, space="PSUM") as avps,
        ):
            # ---- input loads (weights/biases first: the first projection
            # matmuls need wq[0]/xts[0], not the whole xT) -------------------
            wsb = {}
            for name, dram in (("q", wq), ("k", wk), ("v", wv)):
                t = cpool.tile([128, NKT * DQC], BF16, tag=f"w{name}", name=f"w{name}")
                nc.scalar.dma_start(out=t[:, :], in_=dram[:, :])
                wsb[name] = t[:, :].rearrange("p (kt c) -> p kt c", kt=NKT)

            bq_sb = cpool.tile([128, 2], F32, tag="bq")
            nc.scalar.dma_start(out=bq_sb[:, :], in_=bq[:, :])
            bk_sb = cpool.tile([128, 2], F32, tag="bk")
            nc.scalar.dma_start(out=bk_sb[:, :], in_=bk[:, :])
            bv_sb = cpool.tile([1, DQC], F32R, tag="bv")
            nc.scalar.dma_start(out=bv_sb[:, :], in_=bv[:, :])
            ones_row = cpool.tile([1, 128], F32R, tag="ones_row")
            nc.vector.memset(
                ones_row[:, :].bitcast(mybir.dt.uint32), 0x3F800000
            )
            # bias-v broadcast to all partitions via the tensor engine
            bvb_ps = mmps.tile([128, DQC], F32, tag="mm", name="bvb_ps")
            nc.tensor.matmul(
                out=bvb_ps[:, :], lhsT=ones_row[:, :], rhs=bv_sb[:, :],
                start=True, stop=True,
            )
            bvb = cpool.tile([128, DQC], F32, tag="bvb")
            nc.vector.tensor_copy(bvb[:, :], bvb_ps[:, :])

            xts = []
            for kt in range(NKT):
                t = cpool.tile([128, S], BF16, tag=f"xt{kt}", name=f"xt{kt}")
                nc.sync.dma_start(out=t[:, :], in_=xT[kt * 128:(kt + 1) * 128, :])
                xts.append(t)

            # V' tile: [128 (s-chunk partitions), 16 s-chunks x (4 heads x 65)]
            vp = cpool.tile([128, NKC * HPC * VROW], BF16, tag="vp")
            vp4 = vp[:, :].rearrange(
                "p (sc h c) -> p sc h c", sc=NKC, h=HPC, c=VROW
            )
            nc.vector.memset(vp4[:, :, :, DH:VROW], 1.0)

            # ---- qkv^T projection -------------------------------------------
            # Q^T, K^T chunk tiles: [128 (channel), 2048 (s)] x 2 chunks each.
            qth, kth = [None] * HPC, [None] * HPC
            qtc, ktc = [None, None], [None, None]

            def emit_qk_chunk(m):
                for which, chunks, wt, bias in (
                    ("q", qtc, wsb["q"], bq_sb),
                    ("k", ktc, wsb["k"], bk_sb),
                ):
                    chunk = cpool.tile([128, S], BF16, tag=f"{which}tc{m}", name=f"{which}tc{m}")
                    chunks[m] = chunk
                    for qb in range(NQB):
                        ps = mmps.tile([128, 512], F32, tag="mm", name=f"qkps_{which}{m}_{qb}")
                        for kt in range(NKT):
                            nc.tensor.matmul(
                                out=ps[:, :],
                                lhsT=wt[:, kt, m * 128:(m + 1) * 128],
                                rhs=xts[kt][:, qb * 512:(qb + 1) * 512],
                                start=(kt == 0),
                                stop=(kt == NKT - 1),
                            )
                        nc.vector.tensor_scalar(
                            out=chunk[:, qb * 512:(qb + 1) * 512],
                            in0=ps[:, :],
                            scalar1=bias[:, m:m + 1],
                            scalar2=None,
                            op0=ADD,
                        )
                # even head: direct slice; odd head: SBUF->SBUF DMA down to
                # partition base 0 (base-64 matmul APs would engage the
                # row-tiling encoding)
                qth[2 * m] = qtc[m][0:64, :]
                kth[2 * m] = ktc[m][0:64, :]
                h = 2 * m + 1
                qo = cpool.tile([64, S], BF16, tag=f"qto{h}", name=f"qto{h}")
                nc.sync.dma_start(out=qo[:, :], in_=qtc[m][64:128, :])
                qth[h] = qo[:, :]
                ko = cpool.tile([64, S], BF16, tag=f"kto{h}", name=f"kto{h}")
                nc.sync.dma_start(out=ko[:, :], in_=ktc[m][64:128, :])
                kth[h] = ko[:, :]

            def emit_v_chunk(sc):
                # V (+bias) in [s, channel] layout, strided into V' groups.
                ps = mmps.tile([128, DQC], F32, tag="mm", name=f"vps_{sc}")
                for kt in range(NKT):
                    nc.tensor.matmul(
                        out=ps[:, :],
                        lhsT=xts[kt][:, sc * 128:(sc + 1) * 128],
                        rhs=wsb["v"][:, kt, :],
                        start=(kt == 0),
                        stop=(kt == NKT - 1),
                    )
                nc.vector.tensor_tensor(
                    out=vp4[:, sc, :, 0:DH],
                    in0=ps[:, :].rearrange("p (h c) -> p h c", h=HPC),
                    in1=bvb[:, :].rearrange("p (h c) -> p h c", h=HPC),
                    op=ADD,
                )

            # ---- attention ---------------------------------------------------
            attn = [[None] * NKC for _ in range(HPC)]
            av_out = [None] * HPC

            def emit_scores(h, kc):
                at = apool.tile([128, S], BF16, tag="attn", name=f"attn{h}_{kc}")
                attn[h][kc] = at
                for half in range(2):
                    ps = mmps.tile([128, 1024], F32, tag="mm", name=f"sps_{h}_{kc}_{half}")
                    for j in range(2):
                        qb = 2 * half + j
                        nc.tensor.matmul(
                            out=ps[:, j * 512:(j + 1) * 512],
                            lhsT=kth[h][:, kc * 128:(kc + 1) * 128],
                            rhs=qth[h][:, qb * 512:(qb + 1) * 512],
                            start=True,
                            stop=True,
                        )
                    nc.scalar.activation(
                        out=at[:, half * 1024:(half + 1) * 1024],
                        in_=ps[:, :],
                        func=mybir.ActivationFunctionType.Exp,
                        scale=1.0 / np.sqrt(DH),
                    )

            def emit_av(h, kc):
                if kc == 0:
                    av_out[h] = avps.tile([VROW, S], F32, tag="av", name=f"av{h}")
                ps = av_out[h]
                for qb in range(NQB):
                    nc.tensor.matmul(
                        out=ps[:, qb * 512:(qb + 1) * 512],
                        lhsT=vp4[:, kc, h, :],
                        rhs=attn[h][kc][:, qb * 512:(qb + 1) * 512],
                        start=(kc == 0),
                        stop=(kc == NKC - 1),
                    )

            def emit_norm(h):
                ps = av_out[h]
                # Copy PSUM -> SBUF first so the single av_ps slot frees as
                # early as possible (the next head's AV matmuls wait on it).
                ot = wpool.tile([VROW, S], F32, tag="out", name=f"ot{h}")
                nc.vector.tensor_copy(ot[:, :], ps[:, :])
                # Reciprocal of the denominators (ones column of V' summed
                # into row DH). A [1, S] reciprocal runs on one DVE lane at
                # ~7.5 cyc/elem (~13us); scatter the row across 64 partitions
                # first so it takes S/DH elems per lane instead.
                rsc = wpool.tile([DH, S // DH], F32, tag="rsc", name=f"rsc{h}")
                nc.sync.dma_start(out=rsc[:, :], in_=ot[DH:VROW, :])
                rscr = wpool.tile([DH, S // DH], F32R, tag="rscr", name=f"rscr{h}")
                with nc.allow_low_precision(reason="f32r feed for broadcast matmul"):
                    nc.vector.reciprocal(out=rscr[:, :], in_=rsc[:, :])
                rrow = wpool.tile([1, S], F32R, tag="rrow", name=f"rrow{h}")
                nc.sync.dma_start(out=rrow[:, :], in_=rscr[:, :])
                # Broadcast the reciprocal row to 64 partitions with the
                # tensor engine; the output reuses the just-freed av_ps slot
                # so the scores psum rotation never waits on this chain.
                rbp = avps.tile([DH, S], F32, tag="av", name=f"rb{h}")
                for qb in range(NQB):
                    nc.tensor.matmul(
                        out=rbp[:, qb * 512:(qb + 1) * 512],
                        lhsT=ones_row[:, 0:DH],
                        rhs=rrow[:, qb * 512:(qb + 1) * 512],
                        start=True,
                        stop=True,
                    )
                nc.vector.tensor_tensor(
                    out=ot[0:DH, :], in0=ot[0:DH, :], in1=rbp[:, :], op=MULT
                )
                nc.sync.dma_start(out=y[h * DH:(h + 1) * DH, :], in_=ot[0:DH, :])

            emit_qk_chunk(0)
            rest = [lambda m=1: emit_qk_chunk(1)] + [
                lambda sc=sc: emit_v_chunk(sc) for sc in range(NKC)
            ]
            for kc in range(NKC):
                emit_scores(0, kc)
                if kc < 2 and rest:
                    rest.pop(0)()
                elif rest:
                    rest.pop(0)()
            while rest:
                rest.pop(0)()

            LAG = 4
            for h in range(1, HPC):
                last = h == HPC - 1
                for kc in range(NKC):
                    emit_scores(h, kc)
                    emit_av(h - 1, kc)
                    if last and kc >= LAG:
                        emit_av(h, kc - LAG)
                emit_norm(h - 1)
            for kc in range(NKC - LAG, NKC):
                emit_av(HPC - 1, kc)
            emit_norm(HPC - 1)

    if split_waits:
        _split_excess_waits(nc)
    return nc


_NC = None


def _get_nc() -> bass.Bass:
    global _NC
    if _NC is None:
        _NC = _build_nc()
    return _NC


def make_in_maps(x, W_qkv, b_qkv):
    x = np.asarray(x, dtype=np.float32)
    W = np.asarray(W_qkv, dtype=np.float32)
    b = np.asarray(b_qkv, dtype=np.float32)
    in_maps = []
    for c in range(8):
        bi, g = divmod(c, 4)
        cols = slice(g * DQC, (g + 1) * DQC)
        xT = np.ascontiguousarray(x[bi].T).astype(NPBF16)
        def pack_w(block):
            # [1024, 256] -> [128 partitions, kt-major 8*256] contiguous
            w = block[:, cols].reshape(NKT, 128, DQC).transpose(1, 0, 2)
            return np.ascontiguousarray(w.reshape(128, NKT * DQC)).astype(NPBF16)

        m = {
            "xT": xT,
            "wq": pack_w(W[:, 0:D]),
            "wk": pack_w(W[:, D:2 * D]),
            "wv": pack_w(W[:, 2 * D:3 * D]),
            "bq": np.ascontiguousarray(b[0:D][cols].reshape(2, 128).T),
            "bk": np.ascontiguousarray(b[D:2 * D][cols].reshape(2, 128).T),
            "bv": b[2 * D:3 * D][cols].reshape(1, DQC).copy(),
        }
        in_maps.append(m)
    return in_maps


def gather_out(results):
    out = np.zeros((2, S, D), np.float32)
    for c in range(8):
        bi, g = divmod(c, 4)
        out[bi, :, g * DQC:(g + 1) * DQC] = np.asarray(
            results[c]["y"], np.float32
        ).T
    return out


def kernel(x, W_qkv, b_qkv):
    nc = _get_nc()
    in_maps = make_in_maps(x, W_qkv, b_qkv)
    trace = bool(int(os.environ.get("BASS_KERNEL_TRACE", "0")))
    res = run_bass_kernel_spmd(nc, in_maps, list(range(8)), trace=trace)
    if trace:
        kernel.last_result = res
    return gather_out(res.results)
